# revision 1
# baseline (speedup 1.0000x reference)
"""Trainium2 Bass kernel for nn_AutoEncoder (6-layer GCN autoencoder).

Strategy (8 NeuronCores, SPMD), v2:
  - Destination nodes sharded across cores (6250/core, padded to 6272).
  - Node features replicated per layer via AllGather into a padded fp16
    [8*6272, F] node-major DRAM table; per-core dma_gather of h[src] for this
    core's edges (edge lists sorted by local dst window, split into two
    int16-index halves).
  - Segment-sum via one-hot matmuls accumulated in PSUM per 512-dst
    superblock. The one-hot S matrices are HOST-precomputed (fp16, with the
    dst-side deg^-1/2 scale folded into the values) and streamed from DRAM —
    no on-chip one-hot builds.
  - Layer 3 (64-wide input) gathers fp16 node-PAIR rows (256 B) from the
    64-wide table viewed as [NP/2, 128]; parity-split S matrices route each
    edge's correct half, and W3 rows are duplicated so the doubled agg rows
    sum back. All gather tables are therefore uniform 128-wide fp16.
  - The self-loop term never touches the edge path: selfF = y_bn * dinv^2
    stays feature-major and enters the y PSUM via extra fp16 W-matmuls.
  - Everything stays feature-major: BN stats accumulate on ACT during PSUM
    eviction, BN apply (+ReLU) is an ACT pass with per-partition scale/bias,
    and the next-layer node-major table is produced by a DMA xbar transpose
    (no PE transposes, no node-major DVE work).
  - The GCN bias b is skipped (training-mode BatchNorm absorbs it).
"""

import sys

sys.path.insert(0, "/opt/trn_rl_repo")

import numpy as np

N = 50000
E = 800000
F_IN = 128
EPS = 1e-5
NC = 8
SH = 6250  # real dst nodes per core
SHP = 6272  # padded (49 * 128)
NP = NC * SHP  # 50176 rows in the padded replicated node table
HALF = NP // 2  # 25088 (< int16 max) rows per gather table half
WIN = 128  # dst window = psum column band
NWIN = SHP // WIN  # 49
WP = 256  # tile at window-PAIR granularity; scatter matmuls are 256 wide
NWP = 25  # 24 full pairs + lone window 48
NSB = 13  # psum superblocks: 12 x 512 + 1 x 128
CHUNK = 32  # gather chunk size in tiles of 128 edges (main layers)
CHUNK3 = 16  # layer-3 chunk (S2 tiles are 4x wide there)
DIMS = [(128, 128), (128, 128), (128, 64), (64, 128), (128, 128), (128, 128)]
RELU = [True, True, False, True, True, False]

import os as _os
DEBUG_NL = int(_os.environ.get("DEBUG_NL", "6"))  # layers to run (debug)
TRACE = False  # set by test.py for profiling runs
TRACE_KW = {}
LAST_RESULT = None  # BassKernelResults of the last run (for test.py)


def _prep_edges(src_remap, dstl):
    """Per-core edge prep: sort by dst; per-(window, half) edge lists."""
    order = np.argsort(dstl, kind="stable")
    dstl = dstl[order]
    srcr = src_remap[order]
    half = (srcr >= HALF).astype(np.int64)
    w = dstl // WP
    rel = dstl - w * WP
    ed = [[None, None] for _ in range(NWP)]
    for wi in range(NWP):
        m = w == wi
        for h in (0, 1):
            mh = m & (half == h)
            # dedupe by src: one gather slot per distinct src in this window
            gu, inv = np.unique(srcr[mh], return_inverse=True)
            ed[wi][h] = (gu, inv, rel[mh])
    return ed


def _chunks_of(T, ck=CHUNK):
    out = []
    t = 0
    while t < T:
        nt = min(ck, T - t)
        out.append((t, nt))
        t += nt
    return out


def _wrap_idx(gidx, chunks):
    """int16 indices -> [128, total/16] wrapped per chunk, tiled 8x."""
    total_cols = len(gidx) // 16
    arr = np.zeros((16, total_cols), np.int16)
    col = 0
    for t0, nt in chunks:
        cidx = gidx[t0 * 128 : (t0 + nt) * 128]
        ncol = len(cidx) // 16
        arr[:, col : col + ncol] = cidx.reshape(ncol, 16).T
        col += ncol
    assert col == total_cols
    return np.tile(arr, (8, 1)).copy()


def _build_program(tiles, t0s, TA, TB):
    from concourse import bacc, mybir, tile

    FP32 = mybir.dt.float32
    FP16 = mybir.dt.float16
    I16 = mybir.dt.int16
    AX = mybir.AxisListType.X
    OP = mybir.AluOpType
    ACTF = mybir.ActivationFunctionType

    nc = bacc.Bacc(None, num_devices=NC, target_bir_lowering=False, debug=False)

    Ts = [TA, TB]

    # ---- parameters ----
    msgs0_d = [
        nc.declare_dram_parameter("msgs0A", [128, TA, 128], FP16, isOutput=False),
        nc.declare_dram_parameter("msgs0B", [128, TB, 128], FP16, isOutput=False),
    ]
    idx_d = [
        nc.declare_dram_parameter("idxA", [128, TA * 8], I16, isOutput=False),
        nc.declare_dram_parameter("idxB", [128, TB * 8], I16, isOutput=False),
    ]
    idx3_d = [
        nc.declare_dram_parameter("idx3A", [128, TA * 8], I16, isOutput=False),
        nc.declare_dram_parameter("idx3B", [128, TB * 8], I16, isOutput=False),
    ]
    s2_d = [
        nc.declare_dram_parameter("s2A", [128, TA, 256], FP16, isOutput=False),
        nc.declare_dram_parameter("s2B", [128, TB, 256], FP16, isOutput=False),
    ]
    s23_d = [
        nc.declare_dram_parameter("s23A", [128, TA, 2, 256], FP16, isOutput=False),
        nc.declare_dram_parameter("s23B", [128, TB, 2, 256], FP16, isOutput=False),
    ]
    dinvh_d = nc.declare_dram_parameter("dinv2Th", [128, SHP], FP16, isOutput=False)
    selfF0_d = nc.declare_dram_parameter("selfF0", [128, SHP], FP16, isOutput=False)
    Wh_d = [
        nc.declare_dram_parameter(f"Wh{j}", [128, DIMS[j][1]], FP16, isOutput=False)
        for j in range(6)
    ]  # W3 is row-duplicated [128, 128] on the host
    W3s_d = nc.declare_dram_parameter("W3s", [64, 128], FP16, isOutput=False)
    gb_d = [
        nc.declare_dram_parameter(f"gb{j}", [128, 2], FP32, isOutput=False)
        for j in range(6)
    ]
    out_d = nc.declare_dram_parameter("out", [128, SHP], FP32, isOutput=True)

    # ---- internal DRAM: collective bounce buffers ----
    ag_in = [
        nc.dram_tensor(f"ag_in{j}", [SHP, DIMS[j][1]], FP16) for j in range(5)
    ]
    ag_out = [
        nc.dram_tensor(f"ag_out{j}", [NP, DIMS[j][1]], FP16, addr_space="Shared")
        for j in range(5)
    ]
    ar_in = [nc.dram_tensor(f"ar_in{j}", [128, 2], FP32) for j in range(6)]
    ar_out = [
        nc.dram_tensor(f"ar_out{j}", [128, 2], FP32, addr_space="Shared")
        for j in range(6)
    ]

    tbls = [None] + ag_out  # layer 0 streams pre-gathered messages instead

    chunks_m = [_chunks_of(TA), _chunks_of(TB)]
    chunks_3 = [_chunks_of(TA, CHUNK3), _chunks_of(TB, CHUNK3)]
    idx_col0_m = [[], []]
    idx_col0_3 = [[], []]
    for h in (0, 1):
        c = 0
        for _, nt in chunks_m[h]:
            idx_col0_m[h].append(c)
            c += nt * 8
        c = 0
        for _, nt in chunks_3[h]:
            idx_col0_3[h].append(c)
            c += nt * 8

    inv_n = 1.0 / float(N)

    with tile.TileContext(nc) as tc:
        with (
            tc.tile_pool(name="res", bufs=1) as res,
            tc.tile_pool(name="msg", bufs=3) as msgp,
            tc.tile_pool(name="s2", bufs=3) as s2p,
            tc.tile_pool(name="small", bufs=2) as small,
            tc.tile_pool(name="big", bufs=1) as big,
            tc.tile_pool(name="fpp", bufs=2) as fpp,
            tc.tile_pool(name="hx", bufs=1) as hxp,
            tc.tile_pool(name="agg_ps", bufs=2, space="PSUM") as aggp,
            tc.tile_pool(name="y_ps", bufs=2, space="PSUM") as yp,
        ):
            # ---- resident loads ----
            idx_t = [res.tile([128, Ts[h] * 8], I16, name=f"idx{h}") for h in (0, 1)]
            idx3_t = [res.tile([128, Ts[h] * 8], I16, name=f"idx3{h}") for h in (0, 1)]
            for h in (0, 1):
                nc.sync.dma_start(idx_t[h][:], idx_d[h][:])
                nc.sync.dma_start(idx3_t[h][:], idx3_d[h][:])
            dinvh_t = res.tile([128, SHP], FP16, name="dinvh")
            nc.sync.dma_start(dinvh_t[:], dinvh_d[:])
            Wh_t = []
            for j in range(6):
                wt = res.tile([128, DIMS[j][1]], FP16, name=f"Wh{j}")
                nc.sync.dma_start(wt[:], Wh_d[j][:])
                Wh_t.append(wt)
            W3s_t = res.tile([64, 128], FP16, name="W3s")
            nc.sync.dma_start(W3s_t[:], W3s_d[:])
            gb_t = []
            for j in range(6):
                gt = res.tile([128, 2], FP32, name=f"gb{j}")
                nc.sync.dma_start(gt[:], gb_d[j][:])
                gb_t.append(gt)
            selfF = res.tile([128, SHP], FP16, name="selfF0")
            nc.sync.dma_start(selfF[:], selfF0_d[:])

            def bn_vec(j, fo, arr_tile):
                """mean/var -> (scale, shift) columns in a [128, 6] tile."""
                vec = small.tile([128, 6], FP32, tag="bnvec", name="vec")
                nc.vector.tensor_scalar(
                    out=vec[0:fo, 0:1], in0=arr_tile[0:fo, 0:1],
                    scalar1=inv_n, scalar2=None, op0=OP.mult,
                )
                nc.vector.tensor_scalar(
                    out=vec[0:fo, 1:2], in0=arr_tile[0:fo, 1:2],
                    scalar1=inv_n, scalar2=None, op0=OP.mult,
                )
                nc.vector.tensor_tensor(
                    vec[0:fo, 2:3], vec[0:fo, 0:1], vec[0:fo, 0:1], op=OP.mult
                )
                nc.vector.tensor_tensor(
                    vec[0:fo, 2:3], vec[0:fo, 1:2], vec[0:fo, 2:3], op=OP.subtract
                )
                nc.vector.tensor_scalar(
                    out=vec[0:fo, 2:3], in0=vec[0:fo, 2:3],
                    scalar1=float(EPS), scalar2=None, op0=OP.add,
                )
                nc.vector.reciprocal(vec[0:fo, 3:4], vec[0:fo, 2:3])
                nc.scalar.activation(vec[0:fo, 3:4], vec[0:fo, 3:4], ACTF.Sqrt)
                nc.vector.tensor_tensor(
                    vec[0:fo, 4:5], gb_t[j][0:fo, 0:1], vec[0:fo, 3:4], op=OP.mult
                )
                nc.vector.tensor_tensor(
                    vec[0:fo, 5:6], vec[0:fo, 0:1], vec[0:fo, 4:5], op=OP.mult
                )
                nc.vector.tensor_tensor(
                    vec[0:fo, 5:6], gb_t[j][0:fo, 1:2], vec[0:fo, 5:6], op=OP.subtract
                )
                return vec

            for j in range(DEBUG_NL):
                fo = DIMS[j][1]
                pair = j == 3
                tbl = tbls[j]
                idxs = idx3_t if pair else idx_t
                if pair:
                    tblv = tbl[:].rearrange("(a b) f -> a (b f)", b=2)
                s2src = s23_d if pair else s2_d

                cur_chunk = [-1, -1]
                msg_tiles = [None, None]
                s2_tiles = [None, None]
                chunks = chunks_3 if pair else chunks_m
                idx_col0 = idx_col0_3 if pair else idx_col0_m

                def ensure_chunk(h, t):
                    k = 0
                    while not (
                        chunks[h][k][0] <= t < chunks[h][k][0] + chunks[h][k][1]
                    ):
                        k += 1
                    if cur_chunk[h] == k:
                        return
                    cur_chunk[h] = k
                    t0c, ntc = chunks[h][k]
                    mt = msgp.tile([128, ntc, 128], FP16, tag="msg", name="msg")
                    if j == 0:
                        nc.sync.dma_start(
                            mt[:], msgs0_d[h][:, t0c : t0c + ntc, :]
                        )
                    else:
                        in_ap = tblv if pair else tbl[h * HALF : (h + 1) * HALF, :]
                        nc.gpsimd.dma_gather(
                            out_ap=mt[:],
                            in_ap=in_ap,
                            idxs_ap=idxs[h][
                                :, idx_col0[h][k] : idx_col0[h][k] + ntc * 8
                            ],
                            num_idxs=ntc * 128,
                            num_idxs_reg=ntc * 128,
                            elem_size=128,
                            single_packet=False,
                        )
                    msg_tiles[h] = (t0c, mt)
                    if pair:
                        st = s2p.tile([128, ntc, 2, 256], FP16, tag="s2", name="s2")
                        nc.sync.dma_start(st[:], s2src[h][:, t0c : t0c + ntc, :, :])
                    else:
                        st = s2p.tile([128, ntc, 256], FP16, tag="s2", name="s2")
                        nc.sync.dma_start(st[:], s2src[h][:, t0c : t0c + ntc, :])
                    s2_tiles[h] = (t0c, st)

                y_sb = big.tile([128, SHP], FP16, tag="ysb", name="ysb")
                sumP = small.tile([128, NSB], FP32, tag="sumP", name="sumP")
                sqP = small.tile([128, NSB], FP32, tag="sqP", name="sqP")
                junk = small.tile([128, 512], FP16, tag="junk", name="junk")

                for sb in range(NSB):
                    nsb = 512 if sb < 12 else 128
                    wplist = list(range(sb * 2, min(sb * 2 + 2, NWP)))
                    agg = aggp.tile([128, 512], FP32, tag="agg", name="agg")
                    for wp in wplist:
                        woff = (wp % 2) * 256
                        # sequence of (h, t, parity) 256-wide matmuls
                        seq = []
                        for h in (0, 1):
                            for t in range(t0s[wp][h], t0s[wp][h] + tiles[wp][h]):
                                if pair:
                                    seq.append((h, t, 0))
                                    seq.append((h, t, 1))
                                else:
                                    seq.append((h, t, None))
                        fi_eff = 64 if pair else 128
                        for i, (h, t, ps) in enumerate(seq):
                            ensure_chunk(h, t)
                            t0c, mt = msg_tiles[h]
                            s0c, st = s2_tiles[h]
                            if ps is not None:
                                lhsT = mt[:, t - t0c, 64 * ps : 64 * ps + 64]
                                rhs = st[:, t - s0c, ps, :]
                            else:
                                lhsT = mt[:, t - t0c, :]
                                rhs = st[:, t - s0c, :]
                            nc.tensor.matmul(
                                agg[0:fi_eff, woff : woff + 256],
                                lhsT,
                                rhs,
                                start=(i == 0),
                                stop=(i == len(seq) - 1),
                            )
                    # evict agg -> fp16
                    fi_eff = 64 if pair else 128
                    wmain = W3s_t if pair else Wh_t[j]
                    rawT = small.tile([128, 512], FP16, tag="rawT", name="rawT")
                    nc.scalar.activation(
                        rawT[0:fi_eff, 0:nsb], agg[0:fi_eff, 0:nsb], ACTF.Copy
                    )
                    # y = W^T agg + W^T selfF  (fp16 matmuls, f32 psum)
                    c0 = sb * 512
                    y_ps = yp.tile([128, 512], FP32, tag="yps", name="yps")
                    nc.tensor.matmul(
                        y_ps[0:fo, 0:nsb],
                        wmain[0:fi_eff, 0:fo],
                        rawT[0:fi_eff, 0:nsb],
                        start=True,
                        stop=False,
                    )
                    nc.tensor.matmul(
                        y_ps[0:fo, 0:nsb],
                        wmain[0:fi_eff, 0:fo],
                        selfF[0:fi_eff, c0 : c0 + nsb],
                        start=False,
                        stop=True,
                    )
                    # copy to y_sb + stats over valid columns
                    nv = 512 if sb < 12 else 106
                    nc.scalar.activation(
                        y_sb[0:fo, c0 : c0 + nv],
                        y_ps[0:fo, 0:nv],
                        ACTF.Copy,
                        accum_out=sumP[0:fo, sb : sb + 1],
                    )
                    if sb == 12:
                        nc.scalar.activation(
                            y_sb[0:fo, c0 + 106 : c0 + 128],
                            y_ps[0:fo, 106:128],
                            ACTF.Copy,
                        )
                    nc.scalar.activation(
                        junk[0:fo, 0:nv],
                        y_ps[0:fo, 0:nv],
                        ACTF.Square,
                        accum_out=sqP[0:fo, sb : sb + 1],
                    )

                # ---- kick BN stats all-reduce ----
                stats = small.tile([128, 2], FP32, tag="stats", name="stats")
                nc.vector.memset(stats[:], 0.0)
                nc.vector.reduce_sum(stats[0:fo, 0:1], sumP[0:fo, :], axis=AX)
                nc.vector.reduce_sum(stats[0:fo, 1:2], sqP[0:fo, :], axis=AX)
                nc.sync.dma_start(ar_in[j][:], stats[:])
                nc.gpsimd.collective_compute(
                    "AllReduce",
                    OP.add,
                    replica_groups=[list(range(NC))],
                    ins=[ar_in[j][:]],
                    outs=[ar_out[j][:]],
                )
                arr = small.tile([128, 2], FP32, tag="arr", name="arr")
                nc.sync.dma_start(arr[:], ar_out[j][:])
                vec = bn_vec(j, fo, arr)

                if j == 5 or j == DEBUG_NL - 1:
                    # final layer: BN apply fp16 -> f32 staging chunks, DMA out
                    for sb in range(NSB):
                        nsb = 512 if sb < 12 else 128
                        c0 = sb * 512
                        stg = small.tile([128, 512], FP32, tag="ostg", name="ostg")
                        nc.scalar.activation(
                            stg[0:fo, 0:nsb],
                            y_sb[0:fo, c0 : c0 + nsb],
                            ACTF.Identity,
                            bias=vec[0:fo, 5:6],
                            scale=vec[0:fo, 4:5],
                        )
                        nc.sync.dma_start(
                            out_d[:, c0 : c0 + nsb], stg[:, 0:nsb]
                        )
                    continue

                # ---- BN apply (+ReLU) feature-major -> fp16 ----
                y_bnh = fpp.tile([128, SHP], FP16, tag="fp", name="ybnh")
                fn = ACTF.Relu if RELU[j] else ACTF.Identity
                for sb in range(NSB):
                    nsb = 512 if sb < 12 else 128
                    c0 = sb * 512
                    nc.scalar.activation(
                        y_bnh[0:fo, c0 : c0 + nsb],
                        y_sb[0:fo, c0 : c0 + nsb],
                        fn,
                        bias=vec[0:fo, 5:6],
                        scale=vec[0:fo, 4:5],
                    )
                # selfF_next = y_bn * dinv^2 (src scale is folded into S2)
                selfF_next = fpp.tile([128, SHP], FP16, tag="fp", name="selfFn")
                nc.vector.tensor_tensor(
                    selfF_next[0:fo, :], y_bnh[0:fo, :], dinvh_t[0:fo, :], op=OP.mult
                )
                # xbar transpose -> node-major [128, NWIN, fo]
                hnext = hxp.tile([128, NWIN, fo], FP16, tag="hx", name="hnext")
                nc.sync.dma_start_transpose(hnext[:], y_bnh[0:fo, :])
                nc.sync.dma_start(
                    ag_in[j][:].rearrange("(b p) f -> p b f", p=128), hnext[:]
                )
                nc.gpsimd.collective_compute(
                    "AllGather",
                    OP.bypass,
                    replica_groups=[list(range(NC))],
                    ins=[ag_in[j][:]],
                    outs=[ag_out[j][:]],
                )
                selfF = selfF_next

    nc.compile()
    return nc


def kernel(x, edge_index, **params):
    global LAST_RESULT

    from concourse.bass_utils import run_bass_kernel_spmd

    x = np.asarray(x, np.float32)
    edge_index = np.asarray(edge_index, np.int64)
    src_all = edge_index[0]
    dst_all = edge_index[1]

    deg = (np.bincount(dst_all, minlength=N) + 1.0).astype(np.float32)
    dinv = (1.0 / np.sqrt(deg)).astype(np.float32)

    # padded-layout helpers
    remap = (src_all // SH) * SHP + (src_all % SH)

    # padded raw x (layer-0 messages; src scale lives in S2) + x*dinv for selfF0
    hs0 = np.zeros((NP, F_IN), np.float32)
    x_pad_h = np.zeros((NP, F_IN), np.float16)
    xs = x * dinv[:, None]
    dinvp = np.zeros(NP, np.float32)  # dinv per padded id
    for c in range(NC):
        hs0[c * SHP : c * SHP + SH] = xs[c * SH : (c + 1) * SH]
        x_pad_h[c * SHP : c * SHP + SH] = x[c * SH : (c + 1) * SH]
        dinvp[c * SHP : c * SHP + SH] = dinv[c * SH : (c + 1) * SH]

    # per-core edge lists
    eds = []
    dinv_dst = []
    for c in range(NC):
        m = (dst_all >= c * SH) & (dst_all < (c + 1) * SH)
        dstl = dst_all[m] - c * SH
        srcr = remap[m]
        eds.append(_prep_edges(srcr, dstl))
        dv = np.zeros(SHP, np.float32)
        dv[:SH] = dinv[c * SH : (c + 1) * SH]
        dinv_dst.append(dv)

    tiles = [[0, 0] for _ in range(NWP)]
    for w in range(NWP):
        for h in (0, 1):
            mx = max(len(eds[c][w][h][0]) for c in range(NC))
            tiles[w][h] = -(-mx // 128) if mx else 0
    t0s = [[0, 0] for _ in range(NWP)]
    ta = tb = 0
    for w in range(NWP):
        t0s[w][0] = ta
        ta += tiles[w][0]
        t0s[w][1] = tb
        tb += tiles[w][1]
    TA, TB = ta, tb
    Ts = [TA, TB]

    chunksM = [_chunks_of(TA), _chunks_of(TB)]
    chunks3 = [_chunks_of(TA, CHUNK3), _chunks_of(TB, CHUNK3)]

    in_maps = []
    for c in range(NC):
        # build per-half packed streams
        idx_h, idx3_h, s2_h, s23_h, m0_h = [], [], [], [], []
        for h in (0, 1):
            T = Ts[h]
            gidx = np.zeros(T * 128, np.int16)  # half-local src id
            gidx3 = np.zeros(T * 128, np.int16)  # pair id (global)
            s2f = np.zeros((T * 128, 256), np.float32)
            s23f = np.zeros((T * 128, 2, 256), np.float32)
            m0 = np.zeros((T * 128, 128), np.float16)
            for w in range(NWP):
                gu, inv, r = eds[c][w][h]
                nt = tiles[w][h]
                n = len(gu)
                assert n <= nt * 128
                base = t0s[w][h] * 128
                rows = base + np.arange(n)
                gidx[rows] = (gu - h * HALF).astype(np.int16)
                gidx3[rows] = (gu // 2).astype(np.int16)
                vals = dinv_dst[c][w * WP + r] * dinvp[gu[inv]]
                erows = base + inv  # per-edge slot row
                np.add.at(s2f, (erows, r), vals)
                np.add.at(s23f, (erows, (gu[inv] % 2), r), vals)
                m0[rows] = x_pad_h[gu]
            idx_h.append(_wrap_idx(gidx, chunksM[h]))
            idx3_h.append(_wrap_idx(gidx3, chunks3[h]))
            # pre-wrapped layouts [128, T, ...]: partition = slot in tile
            s2_h.append(
                np.ascontiguousarray(
                    s2f.astype(np.float16).reshape(T, 128, 256).transpose(1, 0, 2)
                )
            )
            s23_h.append(
                np.ascontiguousarray(
                    s23f.astype(np.float16)
                    .reshape(T, 128, 2, 256)
                    .transpose(1, 0, 2, 3)
                )
            )
            m0_h.append(
                np.ascontiguousarray(m0.reshape(T, 128, 128).transpose(1, 0, 2))
            )

        dinvT = dinv_dst[c]
        dinv2Th = np.broadcast_to(
            (dinvT * dinvT).astype(np.float16), (128, SHP)
        ).copy()
        own = hs0[c * SHP : (c + 1) * SHP]  # [SHP, F] f32 (= x*dinv)
        selfF0 = (own * dinvT[:, None]).T.astype(np.float16).copy()  # [F, SHP]

        im = {
            "msgs0A": m0_h[0],
            "msgs0B": m0_h[1],
            "idxA": idx_h[0],
            "idxB": idx_h[1],
            "idx3A": idx3_h[0],
            "idx3B": idx3_h[1],
            "s2A": s2_h[0],
            "s2B": s2_h[1],
            "s23A": s23_h[0],
            "s23B": s23_h[1],
            "dinv2Th": dinv2Th,
            "selfF0": selfF0,
        }
        for j in range(6):
            W = np.asarray(params[f"W{j}"], np.float32)
            if j == 3:
                Wd = np.vstack([W, W])  # [128, 128]
            else:
                Wd = W
                if Wd.shape[0] < 128:
                    Wd = np.vstack([Wd, np.zeros((128 - Wd.shape[0], Wd.shape[1]), np.float32)])
            im[f"Wh{j}"] = Wd.astype(np.float16)
            gb = np.zeros((128, 2), np.float32)
            fo = DIMS[j][1]
            gb[:fo, 0] = np.asarray(params[f"g{j}"], np.float32)
            gb[:fo, 1] = np.asarray(params[f"be{j}"], np.float32)
            im[f"gb{j}"] = gb
        im["W3s"] = np.asarray(params["W3"], np.float32).astype(np.float16)
        in_maps.append(im)

    nc = _build_program(tiles, t0s, TA, TB)
    res = run_bass_kernel_spmd(
        nc,
        in_maps,
        core_ids=list(range(NC)),
        trace=TRACE,
        **TRACE_KW,
    )
    LAST_RESULT = res

    out = np.empty((N, F_IN), np.float32)
    for c in range(NC):
        out[c * SH : (c + 1) * SH] = res.results[c]["out"].T[:SH]
    return out



# revision 3
# speedup vs baseline: 1.7059x; 1.7059x over previous
"""Trainium2 Bass kernel for nn_AutoEncoder (6-layer GCN autoencoder).

Strategy (8 NeuronCores, SPMD), v2:
  - Destination nodes sharded across cores (6250/core, padded to 6272).
  - Node features replicated per layer via AllGather into a padded fp16
    [8*6272, F] node-major DRAM table; per-core dma_gather of h[src] for this
    core's edges (edge lists sorted by local dst window, split into two
    int16-index halves).
  - Segment-sum via one-hot matmuls accumulated in PSUM per 512-dst
    superblock. The one-hot S matrices are HOST-precomputed (fp16, with the
    dst-side deg^-1/2 scale folded into the values) and streamed from DRAM —
    no on-chip one-hot builds.
  - Layer 3 (64-wide input) gathers fp16 node-PAIR rows (256 B) from the
    64-wide table viewed as [NP/2, 128]; parity-split S matrices route each
    edge's correct half, and W3 rows are duplicated so the doubled agg rows
    sum back. All gather tables are therefore uniform 128-wide fp16.
  - The self-loop term never touches the edge path: selfF = y_bn * dinv^2
    stays feature-major and enters the y PSUM via extra fp16 W-matmuls.
  - Everything stays feature-major: BN stats accumulate on ACT during PSUM
    eviction, BN apply (+ReLU) is an ACT pass with per-partition scale/bias,
    and the next-layer node-major table is produced by a DMA xbar transpose
    (no PE transposes, no node-major DVE work).
  - The GCN bias b is skipped (training-mode BatchNorm absorbs it).
"""

import sys

sys.path.insert(0, "/opt/trn_rl_repo")

import numpy as np

N = 50000
E = 800000
F_IN = 128
EPS = 1e-5
NC = 8
SH = 6250  # real dst nodes per core
SHP = 6272  # padded (49 * 128)
NP = NC * SHP  # 50176 rows in the padded replicated node table
HALF = NP // 2  # 25088 (< int16 max) rows per gather table half
WIN = 128  # dst window = psum column band
NWIN = SHP // WIN  # 49
WP = 256  # tile at window-PAIR granularity; scatter matmuls are 256 wide
NWP = 25  # 24 full pairs + lone window 48
NSB = 13  # psum superblocks: 12 x 512 + 1 x 128
CHUNK = 32  # gather chunk size in tiles of 128 edges (main layers)
CHUNK3 = 16  # layer-3 chunk (S2 tiles are 4x wide there)
DIMS = [(128, 128), (128, 128), (128, 64), (64, 128), (128, 128), (128, 128)]
RELU = [True, True, False, True, True, False]

import os as _os
DEBUG_NL = int(_os.environ.get("DEBUG_NL", "6"))  # layers to run (debug)
TRACE = False  # set by test.py for profiling runs
TRACE_KW = {}
LAST_RESULT = None  # BassKernelResults of the last run (for test.py)


def _prep_edges(src_remap, dstl):
    """Per-core edge prep: sort by dst; per-(window, half) edge lists."""
    order = np.argsort(dstl, kind="stable")
    dstl = dstl[order]
    srcr = src_remap[order]
    half = (srcr >= HALF).astype(np.int64)
    w = dstl // WP
    rel = dstl - w * WP
    ed = [[None, None] for _ in range(NWP)]
    for wi in range(NWP):
        m = w == wi
        for h in (0, 1):
            mh = m & (half == h)
            # dedupe by src: one gather slot per distinct src in this window
            gu, inv = np.unique(srcr[mh], return_inverse=True)
            ed[wi][h] = (gu, inv, rel[mh])
    return ed


def _chunks_of(T, ck=CHUNK):
    out = []
    t = 0
    while t < T:
        nt = min(ck, T - t)
        out.append((t, nt))
        t += nt
    return out


def _wrap_idx(gidx, chunks):
    """int16 indices -> [128, total/16] wrapped per chunk, tiled 8x."""
    total_cols = len(gidx) // 16
    arr = np.zeros((16, total_cols), np.int16)
    col = 0
    for t0, nt in chunks:
        cidx = gidx[t0 * 128 : (t0 + nt) * 128]
        ncol = len(cidx) // 16
        arr[:, col : col + ncol] = cidx.reshape(ncol, 16).T
        col += ncol
    assert col == total_cols
    return np.tile(arr, (8, 1)).copy()


def _build_program(tiles, t0s, TA, TB):
    from concourse import bacc, mybir, tile

    FP32 = mybir.dt.float32
    FP16 = mybir.dt.float16
    I16 = mybir.dt.int16
    AX = mybir.AxisListType.X
    OP = mybir.AluOpType
    ACTF = mybir.ActivationFunctionType

    nc = bacc.Bacc(None, num_devices=NC, target_bir_lowering=False, debug=False, num_swdge_queues=4)

    Ts = [TA, TB]

    # ---- parameters ----
    msgs0_d = [
        nc.declare_dram_parameter("msgs0A", [128, TA, 128], FP16, isOutput=False),
        nc.declare_dram_parameter("msgs0B", [128, TB, 128], FP16, isOutput=False),
    ]
    idx_d = [
        nc.declare_dram_parameter("idxA", [128, TA * 8], I16, isOutput=False),
        nc.declare_dram_parameter("idxB", [128, TB * 8], I16, isOutput=False),
    ]
    idx3_d = [
        nc.declare_dram_parameter("idx3A", [128, TA * 8], I16, isOutput=False),
        nc.declare_dram_parameter("idx3B", [128, TB * 8], I16, isOutput=False),
    ]
    s2_d = [
        nc.declare_dram_parameter("s2A", [128, TA, 256], FP16, isOutput=False),
        nc.declare_dram_parameter("s2B", [128, TB, 256], FP16, isOutput=False),
    ]
    s23_d = [
        nc.declare_dram_parameter("s23A", [128, TA, 2, 256], FP16, isOutput=False),
        nc.declare_dram_parameter("s23B", [128, TB, 2, 256], FP16, isOutput=False),
    ]
    dinvh_d = nc.declare_dram_parameter("dinv2Th", [128, SHP], FP16, isOutput=False)
    selfF0_d = nc.declare_dram_parameter("selfF0", [128, SHP], FP16, isOutput=False)
    Wh_d = [
        nc.declare_dram_parameter(f"Wh{j}", [128, DIMS[j][1]], FP16, isOutput=False)
        for j in range(6)
    ]  # W3 is row-duplicated [128, 128] on the host
    W3s_d = nc.declare_dram_parameter("W3s", [64, 128], FP16, isOutput=False)
    gb_d = [
        nc.declare_dram_parameter(f"gb{j}", [128, 2], FP32, isOutput=False)
        for j in range(6)
    ]
    out_d = nc.declare_dram_parameter("out", [128, SHP], FP32, isOutput=True)

    # ---- internal DRAM: collective bounce buffers ----
    ag_in = [
        nc.dram_tensor(f"ag_in{j}", [SHP, DIMS[j][1]], FP16) for j in range(5)
    ]
    ag_out = [
        nc.dram_tensor(f"ag_out{j}", [NP, DIMS[j][1]], FP16, addr_space="Shared")
        for j in range(5)
    ]
    ar_in = [nc.dram_tensor(f"ar_in{j}", [128, 2], FP32) for j in range(6)]
    ar_out = [
        nc.dram_tensor(f"ar_out{j}", [128, 2], FP32, addr_space="Shared")
        for j in range(6)
    ]

    tbls = [None] + ag_out  # layer 0 streams pre-gathered messages instead

    chunks_m = [_chunks_of(TA), _chunks_of(TB)]
    chunks_3 = [_chunks_of(TA, CHUNK3), _chunks_of(TB, CHUNK3)]
    idx_col0_m = [[], []]
    idx_col0_3 = [[], []]
    for h in (0, 1):
        c = 0
        for _, nt in chunks_m[h]:
            idx_col0_m[h].append(c)
            c += nt * 8
        c = 0
        for _, nt in chunks_3[h]:
            idx_col0_3[h].append(c)
            c += nt * 8

    inv_n = 1.0 / float(N)

    with tile.TileContext(nc) as tc:
        with (
            tc.tile_pool(name="res", bufs=1) as res,
            tc.tile_pool(name="msg", bufs=5) as msgp,
            tc.tile_pool(name="s2", bufs=3) as s2p,
            tc.tile_pool(name="small", bufs=2) as small,
            tc.tile_pool(name="big", bufs=1) as big,
            tc.tile_pool(name="fpp", bufs=2) as fpp,
            tc.tile_pool(name="hx", bufs=1) as hxp,
            tc.tile_pool(name="agg_ps", bufs=2, space="PSUM") as aggp,
            tc.tile_pool(name="y_ps", bufs=2, space="PSUM") as yp,
        ):
            # ---- resident loads ----
            idx_t = [res.tile([128, Ts[h] * 8], I16, name=f"idx{h}") for h in (0, 1)]
            idx3_t = [res.tile([128, Ts[h] * 8], I16, name=f"idx3{h}") for h in (0, 1)]
            for h in (0, 1):
                nc.sync.dma_start(idx_t[h][:], idx_d[h][:])
                nc.sync.dma_start(idx3_t[h][:], idx3_d[h][:])
            dinvh_t = res.tile([128, SHP], FP16, name="dinvh")
            nc.sync.dma_start(dinvh_t[:], dinvh_d[:])
            Wh_t = []
            for j in range(6):
                wt = res.tile([128, DIMS[j][1]], FP16, name=f"Wh{j}")
                nc.sync.dma_start(wt[:], Wh_d[j][:])
                Wh_t.append(wt)
            W3s_t = res.tile([64, 128], FP16, name="W3s")
            nc.sync.dma_start(W3s_t[:], W3s_d[:])
            gb_t = []
            for j in range(6):
                gt = res.tile([128, 2], FP32, name=f"gb{j}")
                nc.sync.dma_start(gt[:], gb_d[j][:])
                gb_t.append(gt)
            selfF = res.tile([128, SHP], FP16, name="selfF0")
            nc.sync.dma_start(selfF[:], selfF0_d[:])

            def bn_vec(j, fo, arr_tile):
                """mean/var -> (scale, shift) columns in a [128, 6] tile."""
                vec = small.tile([128, 6], FP32, tag="bnvec", name="vec")
                nc.vector.tensor_scalar(
                    out=vec[0:fo, 0:1], in0=arr_tile[0:fo, 0:1],
                    scalar1=inv_n, scalar2=None, op0=OP.mult,
                )
                nc.vector.tensor_scalar(
                    out=vec[0:fo, 1:2], in0=arr_tile[0:fo, 1:2],
                    scalar1=inv_n, scalar2=None, op0=OP.mult,
                )
                nc.vector.tensor_tensor(
                    vec[0:fo, 2:3], vec[0:fo, 0:1], vec[0:fo, 0:1], op=OP.mult
                )
                nc.vector.tensor_tensor(
                    vec[0:fo, 2:3], vec[0:fo, 1:2], vec[0:fo, 2:3], op=OP.subtract
                )
                nc.vector.tensor_scalar(
                    out=vec[0:fo, 2:3], in0=vec[0:fo, 2:3],
                    scalar1=float(EPS), scalar2=None, op0=OP.add,
                )
                nc.vector.reciprocal(vec[0:fo, 3:4], vec[0:fo, 2:3])
                nc.scalar.activation(vec[0:fo, 3:4], vec[0:fo, 3:4], ACTF.Sqrt)
                nc.vector.tensor_tensor(
                    vec[0:fo, 4:5], gb_t[j][0:fo, 0:1], vec[0:fo, 3:4], op=OP.mult
                )
                nc.vector.tensor_tensor(
                    vec[0:fo, 5:6], vec[0:fo, 0:1], vec[0:fo, 4:5], op=OP.mult
                )
                nc.vector.tensor_tensor(
                    vec[0:fo, 5:6], gb_t[j][0:fo, 1:2], vec[0:fo, 5:6], op=OP.subtract
                )
                return vec

            gq_ctr = [0]

            for j in range(DEBUG_NL):
                fo = DIMS[j][1]
                pair = j == 3
                tbl = tbls[j]
                idxs = idx3_t if pair else idx_t
                if pair:
                    tblv = tbl[:].rearrange("(a b) f -> a (b f)", b=2)
                s2src = s23_d if pair else s2_d

                cur_chunk = [-1, -1]
                msg_tiles = [None, None]
                s2_tiles = [None, None]
                chunks = chunks_3 if pair else chunks_m
                idx_col0 = idx_col0_3 if pair else idx_col0_m

                def ensure_chunk(h, t):
                    k = 0
                    while not (
                        chunks[h][k][0] <= t < chunks[h][k][0] + chunks[h][k][1]
                    ):
                        k += 1
                    if cur_chunk[h] == k:
                        return
                    cur_chunk[h] = k
                    t0c, ntc = chunks[h][k]
                    mt = msgp.tile([128, ntc, 128], FP16, tag="msg", name="msg")
                    if j == 0:
                        nc.sync.dma_start(
                            mt[:], msgs0_d[h][:, t0c : t0c + ntc, :]
                        )
                    else:
                        in_ap = tblv if pair else tbl[h * HALF : (h + 1) * HALF, :]
                        nc.gpsimd.dma_gather(
                            out_ap=mt[:],
                            in_ap=in_ap,
                            idxs_ap=idxs[h][
                                :, idx_col0[h][k] : idx_col0[h][k] + ntc * 8
                            ],
                            num_idxs=ntc * 128,
                            num_idxs_reg=ntc * 128,
                            elem_size=128,
                            single_packet=False,
                            queue_num=gq_ctr[0] % 4,
                        )
                        gq_ctr[0] += 1
                    msg_tiles[h] = (t0c, mt)
                    if pair:
                        st = s2p.tile([128, ntc, 2, 256], FP16, tag="s2", name="s2")
                        nc.sync.dma_start(st[:], s2src[h][:, t0c : t0c + ntc, :, :])
                    else:
                        st = s2p.tile([128, ntc, 256], FP16, tag="s2", name="s2")
                        nc.sync.dma_start(st[:], s2src[h][:, t0c : t0c + ntc, :])
                    s2_tiles[h] = (t0c, st)

                y_sb = big.tile([128, SHP], FP16, tag="ysb", name="ysb")
                sumP = small.tile([128, NSB], FP32, tag="sumP", name="sumP")
                sqP = small.tile([128, NSB], FP32, tag="sqP", name="sqP")
                junk = small.tile([128, 512], FP16, tag="junk", name="junk")

                for sb in range(NSB):
                    nsb = 512 if sb < 12 else 128
                    wplist = list(range(sb * 2, min(sb * 2 + 2, NWP)))
                    agg = aggp.tile([128, 512], FP32, tag="agg", name="agg")
                    for wp in wplist:
                        woff = (wp % 2) * 256
                        # sequence of (h, t, parity) 256-wide matmuls
                        seq = []
                        for h in (0, 1):
                            for t in range(t0s[wp][h], t0s[wp][h] + tiles[wp][h]):
                                if pair:
                                    seq.append((h, t, 0))
                                    seq.append((h, t, 1))
                                else:
                                    seq.append((h, t, None))
                        fi_eff = 64 if pair else 128
                        for i, (h, t, ps) in enumerate(seq):
                            ensure_chunk(h, t)
                            t0c, mt = msg_tiles[h]
                            s0c, st = s2_tiles[h]
                            if ps is not None:
                                lhsT = mt[:, t - t0c, 64 * ps : 64 * ps + 64]
                                rhs = st[:, t - s0c, ps, :]
                            else:
                                lhsT = mt[:, t - t0c, :]
                                rhs = st[:, t - s0c, :]
                            nc.tensor.matmul(
                                agg[0:fi_eff, woff : woff + 256],
                                lhsT,
                                rhs,
                                start=(i == 0),
                                stop=(i == len(seq) - 1),
                            )
                    # evict agg -> fp16
                    fi_eff = 64 if pair else 128
                    wmain = W3s_t if pair else Wh_t[j]
                    rawT = small.tile([128, 512], FP16, tag="rawT", name="rawT")
                    nc.scalar.activation(
                        rawT[0:fi_eff, 0:nsb], agg[0:fi_eff, 0:nsb], ACTF.Copy
                    )
                    # y = W^T agg + W^T selfF  (fp16 matmuls, f32 psum)
                    c0 = sb * 512
                    y_ps = yp.tile([128, 512], FP32, tag="yps", name="yps")
                    nc.tensor.matmul(
                        y_ps[0:fo, 0:nsb],
                        wmain[0:fi_eff, 0:fo],
                        rawT[0:fi_eff, 0:nsb],
                        start=True,
                        stop=False,
                    )
                    nc.tensor.matmul(
                        y_ps[0:fo, 0:nsb],
                        wmain[0:fi_eff, 0:fo],
                        selfF[0:fi_eff, c0 : c0 + nsb],
                        start=False,
                        stop=True,
                    )
                    # copy to y_sb + stats over valid columns
                    nv = 512 if sb < 12 else 106
                    nc.scalar.activation(
                        y_sb[0:fo, c0 : c0 + nv],
                        y_ps[0:fo, 0:nv],
                        ACTF.Copy,
                        accum_out=sumP[0:fo, sb : sb + 1],
                    )
                    if sb == 12:
                        nc.scalar.activation(
                            y_sb[0:fo, c0 + 106 : c0 + 128],
                            y_ps[0:fo, 106:128],
                            ACTF.Copy,
                        )
                    nc.scalar.activation(
                        junk[0:fo, 0:nv],
                        y_ps[0:fo, 0:nv],
                        ACTF.Square,
                        accum_out=sqP[0:fo, sb : sb + 1],
                    )

                # ---- kick BN stats all-reduce ----
                stats = small.tile([128, 2], FP32, tag="stats", name="stats")
                nc.vector.memset(stats[:], 0.0)
                nc.vector.reduce_sum(stats[0:fo, 0:1], sumP[0:fo, :], axis=AX)
                nc.vector.reduce_sum(stats[0:fo, 1:2], sqP[0:fo, :], axis=AX)
                nc.sync.dma_start(ar_in[j][:], stats[:])
                nc.gpsimd.collective_compute(
                    "AllReduce",
                    OP.add,
                    replica_groups=[list(range(NC))],
                    ins=[ar_in[j][:]],
                    outs=[ar_out[j][:]],
                )
                arr = small.tile([128, 2], FP32, tag="arr", name="arr")
                nc.sync.dma_start(arr[:], ar_out[j][:])
                vec = bn_vec(j, fo, arr)

                if j == 5 or j == DEBUG_NL - 1:
                    # final layer: BN apply fp16 -> f32 staging chunks, DMA out
                    for sb in range(NSB):
                        nsb = 512 if sb < 12 else 128
                        c0 = sb * 512
                        stg = small.tile([128, 512], FP32, tag="ostg", name="ostg")
                        nc.scalar.activation(
                            stg[0:fo, 0:nsb],
                            y_sb[0:fo, c0 : c0 + nsb],
                            ACTF.Identity,
                            bias=vec[0:fo, 5:6],
                            scale=vec[0:fo, 4:5],
                        )
                        nc.sync.dma_start(
                            out_d[:, c0 : c0 + nsb], stg[:, 0:nsb]
                        )
                    continue

                # ---- BN apply (+ReLU) feature-major -> fp16 ----
                y_bnh = fpp.tile([128, SHP], FP16, tag="fp", name="ybnh")
                fn = ACTF.Relu if RELU[j] else ACTF.Identity
                for sb in range(NSB):
                    nsb = 512 if sb < 12 else 128
                    c0 = sb * 512
                    nc.scalar.activation(
                        y_bnh[0:fo, c0 : c0 + nsb],
                        y_sb[0:fo, c0 : c0 + nsb],
                        fn,
                        bias=vec[0:fo, 5:6],
                        scale=vec[0:fo, 4:5],
                    )
                # selfF_next = y_bn * dinv^2 (src scale is folded into S2)
                selfF_next = fpp.tile([128, SHP], FP16, tag="fp", name="selfFn")
                nc.vector.tensor_tensor(
                    selfF_next[0:fo, :], y_bnh[0:fo, :], dinvh_t[0:fo, :], op=OP.mult
                )
                # xbar transpose -> node-major [128, NWIN, fo]
                hnext = hxp.tile([128, NWIN, fo], FP16, tag="hx", name="hnext")
                nc.sync.dma_start_transpose(hnext[:], y_bnh[0:fo, :])
                nc.sync.dma_start(
                    ag_in[j][:].rearrange("(b p) f -> p b f", p=128), hnext[:]
                )
                nc.gpsimd.collective_compute(
                    "AllGather",
                    OP.bypass,
                    replica_groups=[list(range(NC))],
                    ins=[ag_in[j][:]],
                    outs=[ag_out[j][:]],
                )
                selfF = selfF_next

    nc.compile()
    return nc


def kernel(x, edge_index, **params):
    global LAST_RESULT

    from concourse.bass_utils import run_bass_kernel_spmd

    x = np.asarray(x, np.float32)
    edge_index = np.asarray(edge_index, np.int64)
    src_all = edge_index[0]
    dst_all = edge_index[1]

    deg = (np.bincount(dst_all, minlength=N) + 1.0).astype(np.float32)
    dinv = (1.0 / np.sqrt(deg)).astype(np.float32)

    # padded-layout helpers
    remap = (src_all // SH) * SHP + (src_all % SH)

    # padded raw x (layer-0 messages; src scale lives in S2) + x*dinv for selfF0
    hs0 = np.zeros((NP, F_IN), np.float32)
    x_pad_h = np.zeros((NP, F_IN), np.float16)
    xs = x * dinv[:, None]
    dinvp = np.zeros(NP, np.float32)  # dinv per padded id
    for c in range(NC):
        hs0[c * SHP : c * SHP + SH] = xs[c * SH : (c + 1) * SH]
        x_pad_h[c * SHP : c * SHP + SH] = x[c * SH : (c + 1) * SH]
        dinvp[c * SHP : c * SHP + SH] = dinv[c * SH : (c + 1) * SH]

    # per-core edge lists
    eds = []
    dinv_dst = []
    for c in range(NC):
        m = (dst_all >= c * SH) & (dst_all < (c + 1) * SH)
        dstl = dst_all[m] - c * SH
        srcr = remap[m]
        eds.append(_prep_edges(srcr, dstl))
        dv = np.zeros(SHP, np.float32)
        dv[:SH] = dinv[c * SH : (c + 1) * SH]
        dinv_dst.append(dv)

    tiles = [[0, 0] for _ in range(NWP)]
    for w in range(NWP):
        for h in (0, 1):
            mx = max(len(eds[c][w][h][0]) for c in range(NC))
            tiles[w][h] = -(-mx // 128) if mx else 0
    t0s = [[0, 0] for _ in range(NWP)]
    ta = tb = 0
    for w in range(NWP):
        t0s[w][0] = ta
        ta += tiles[w][0]
        t0s[w][1] = tb
        tb += tiles[w][1]
    TA, TB = ta, tb
    Ts = [TA, TB]

    chunksM = [_chunks_of(TA), _chunks_of(TB)]
    chunks3 = [_chunks_of(TA, CHUNK3), _chunks_of(TB, CHUNK3)]

    in_maps = []
    for c in range(NC):
        # build per-half packed streams
        idx_h, idx3_h, s2_h, s23_h, m0_h = [], [], [], [], []
        for h in (0, 1):
            T = Ts[h]
            gidx = np.zeros(T * 128, np.int16)  # half-local src id
            gidx3 = np.zeros(T * 128, np.int16)  # pair id (global)
            s2f = np.zeros((T * 128, 256), np.float32)
            s23f = np.zeros((T * 128, 2, 256), np.float32)
            m0 = np.zeros((T * 128, 128), np.float16)
            for w in range(NWP):
                gu, inv, r = eds[c][w][h]
                nt = tiles[w][h]
                n = len(gu)
                assert n <= nt * 128
                base = t0s[w][h] * 128
                rows = base + np.arange(n)
                gidx[rows] = (gu - h * HALF).astype(np.int16)
                gidx3[rows] = (gu // 2).astype(np.int16)
                vals = dinv_dst[c][w * WP + r] * dinvp[gu[inv]]
                erows = base + inv  # per-edge slot row
                np.add.at(s2f, (erows, r), vals)
                np.add.at(s23f, (erows, (gu[inv] % 2), r), vals)
                m0[rows] = x_pad_h[gu]
            idx_h.append(_wrap_idx(gidx, chunksM[h]))
            idx3_h.append(_wrap_idx(gidx3, chunks3[h]))
            # pre-wrapped layouts [128, T, ...]: partition = slot in tile
            s2_h.append(
                np.ascontiguousarray(
                    s2f.astype(np.float16).reshape(T, 128, 256).transpose(1, 0, 2)
                )
            )
            s23_h.append(
                np.ascontiguousarray(
                    s23f.astype(np.float16)
                    .reshape(T, 128, 2, 256)
                    .transpose(1, 0, 2, 3)
                )
            )
            m0_h.append(
                np.ascontiguousarray(m0.reshape(T, 128, 128).transpose(1, 0, 2))
            )

        dinvT = dinv_dst[c]
        dinv2Th = np.broadcast_to(
            (dinvT * dinvT).astype(np.float16), (128, SHP)
        ).copy()
        own = hs0[c * SHP : (c + 1) * SHP]  # [SHP, F] f32 (= x*dinv)
        selfF0 = (own * dinvT[:, None]).T.astype(np.float16).copy()  # [F, SHP]

        im = {
            "msgs0A": m0_h[0],
            "msgs0B": m0_h[1],
            "idxA": idx_h[0],
            "idxB": idx_h[1],
            "idx3A": idx3_h[0],
            "idx3B": idx3_h[1],
            "s2A": s2_h[0],
            "s2B": s2_h[1],
            "s23A": s23_h[0],
            "s23B": s23_h[1],
            "dinv2Th": dinv2Th,
            "selfF0": selfF0,
        }
        for j in range(6):
            W = np.asarray(params[f"W{j}"], np.float32)
            if j == 3:
                Wd = np.vstack([W, W])  # [128, 128]
            else:
                Wd = W
                if Wd.shape[0] < 128:
                    Wd = np.vstack([Wd, np.zeros((128 - Wd.shape[0], Wd.shape[1]), np.float32)])
            im[f"Wh{j}"] = Wd.astype(np.float16)
            gb = np.zeros((128, 2), np.float32)
            fo = DIMS[j][1]
            gb[:fo, 0] = np.asarray(params[f"g{j}"], np.float32)
            gb[:fo, 1] = np.asarray(params[f"be{j}"], np.float32)
            im[f"gb{j}"] = gb
        im["W3s"] = np.asarray(params["W3"], np.float32).astype(np.float16)
        in_maps.append(im)

    nc = _build_program(tiles, t0s, TA, TB)
    res = run_bass_kernel_spmd(
        nc,
        in_maps,
        core_ids=list(range(NC)),
        trace=TRACE,
        **TRACE_KW,
    )
    LAST_RESULT = res

    out = np.empty((N, F_IN), np.float32)
    for c in range(NC):
        out[c * SH : (c + 1) * SH] = res.results[c]["out"].T[:SH]
    return out



# revision 4
# speedup vs baseline: 1.8238x; 1.0691x over previous
"""Trainium2 Bass kernel for nn_AutoEncoder (6-layer GCN autoencoder).

v2 + multi-queue gathers (HW 3.26 ms, was 5.02 ms): dma_gather descriptor
generation runs on Q7 core pair (2*queue_num, 2*queue_num+1), so rotating
gathers across 4 SWDGE queues overlaps desc-gen ~2.5x. msg pool bufs=5
keeps >=4 gather chunks in flight.

Strategy (8 NeuronCores, SPMD), v2:
  - Destination nodes sharded across cores (6250/core, padded to 6272).
  - Node features replicated per layer via AllGather into a padded fp16
    [8*6272, F] node-major DRAM table; per-core dma_gather of h[src] for this
    core's edges (edge lists sorted by local dst window, split into two
    int16-index halves).
  - Segment-sum via one-hot matmuls accumulated in PSUM per 512-dst
    superblock. The one-hot S matrices are HOST-precomputed (fp16, with the
    dst-side deg^-1/2 scale folded into the values) and streamed from DRAM —
    no on-chip one-hot builds.
  - Layer 3 (64-wide input) gathers fp16 node-PAIR rows (256 B) from the
    64-wide table viewed as [NP/2, 128]; parity-split S matrices route each
    edge's correct half, and W3 rows are duplicated so the doubled agg rows
    sum back. All gather tables are therefore uniform 128-wide fp16.
  - The self-loop term never touches the edge path: selfF = y_bn * dinv^2
    stays feature-major and enters the y PSUM via extra fp16 W-matmuls.
  - Everything stays feature-major: BN stats accumulate on ACT during PSUM
    eviction, BN apply (+ReLU) is an ACT pass with per-partition scale/bias,
    and the next-layer node-major table is produced by a DMA xbar transpose
    (no PE transposes, no node-major DVE work).
  - The GCN bias b is skipped (training-mode BatchNorm absorbs it).
"""

import sys

sys.path.insert(0, "/opt/trn_rl_repo")

import numpy as np

N = 50000
E = 800000
F_IN = 128
EPS = 1e-5
NC = 8
SH = 6250  # real dst nodes per core
SHP = 6272  # padded (49 * 128)
NP = NC * SHP  # 50176 rows in the padded replicated node table
HALF = NP // 2  # 25088 (< int16 max) rows per gather table half
WIN = 128  # dst window = psum column band
NWIN = SHP // WIN  # 49
WP = 256  # tile at window-PAIR granularity; scatter matmuls are 256 wide
NWP = 25  # 24 full pairs + lone window 48
NSB = 13  # psum superblocks: 12 x 512 + 1 x 128
CHUNK = 32  # gather chunk size in tiles of 128 edges (main layers)
CHUNK3 = 16  # layer-3 chunk (S2 tiles are 4x wide there)
DIMS = [(128, 128), (128, 128), (128, 64), (64, 128), (128, 128), (128, 128)]
RELU = [True, True, False, True, True, False]

import os as _os
DEBUG_NL = int(_os.environ.get("DEBUG_NL", "6"))  # layers to run (debug)
TRACE = False  # set by test.py for profiling runs
TRACE_KW = {}
LAST_RESULT = None  # BassKernelResults of the last run (for test.py)


def _prep_edges(src_remap, dstl):
    """Per-core edge prep: sort by dst; per-(window, half) edge lists."""
    order = np.argsort(dstl, kind="stable")
    dstl = dstl[order]
    srcr = src_remap[order]
    half = (srcr >= HALF).astype(np.int64)
    w = dstl // WP
    rel = dstl - w * WP
    ed = [[None, None] for _ in range(NWP)]
    for wi in range(NWP):
        m = w == wi
        for h in (0, 1):
            mh = m & (half == h)
            # dedupe by src: one gather slot per distinct src in this window
            gu, inv = np.unique(srcr[mh], return_inverse=True)
            ed[wi][h] = (gu, inv, rel[mh])
    return ed


def _chunks_of(T, ck=CHUNK):
    out = []
    t = 0
    while t < T:
        nt = min(ck, T - t)
        out.append((t, nt))
        t += nt
    return out


def _wrap_idx(gidx, chunks):
    """int16 indices -> [128, total/16] wrapped per chunk, tiled 8x."""
    total_cols = len(gidx) // 16
    arr = np.zeros((16, total_cols), np.int16)
    col = 0
    for t0, nt in chunks:
        cidx = gidx[t0 * 128 : (t0 + nt) * 128]
        ncol = len(cidx) // 16
        arr[:, col : col + ncol] = cidx.reshape(ncol, 16).T
        col += ncol
    assert col == total_cols
    return np.tile(arr, (8, 1)).copy()


def _build_program(tiles, t0s, TA, TB):
    from concourse import bacc, mybir, tile

    FP32 = mybir.dt.float32
    FP16 = mybir.dt.float16
    I16 = mybir.dt.int16
    AX = mybir.AxisListType.X
    OP = mybir.AluOpType
    ACTF = mybir.ActivationFunctionType

    nc = bacc.Bacc(None, num_devices=NC, target_bir_lowering=False, debug=False, num_swdge_queues=4)

    Ts = [TA, TB]

    # ---- parameters ----
    msgs0_d = [
        nc.declare_dram_parameter("msgs0A", [128, TA, 128], FP16, isOutput=False),
        nc.declare_dram_parameter("msgs0B", [128, TB, 128], FP16, isOutput=False),
    ]
    idx_d = [
        nc.declare_dram_parameter("idxA", [128, TA * 8], I16, isOutput=False),
        nc.declare_dram_parameter("idxB", [128, TB * 8], I16, isOutput=False),
    ]
    idx3_d = [
        nc.declare_dram_parameter("idx3A", [128, TA * 8], I16, isOutput=False),
        nc.declare_dram_parameter("idx3B", [128, TB * 8], I16, isOutput=False),
    ]
    s2_d = [
        nc.declare_dram_parameter("s2A", [128, TA, 256], FP16, isOutput=False),
        nc.declare_dram_parameter("s2B", [128, TB, 256], FP16, isOutput=False),
    ]
    s23_d = [
        nc.declare_dram_parameter("s23A", [128, TA, 2, 256], FP16, isOutput=False),
        nc.declare_dram_parameter("s23B", [128, TB, 2, 256], FP16, isOutput=False),
    ]
    dinvh_d = nc.declare_dram_parameter("dinv2Th", [128, SHP], FP16, isOutput=False)
    selfF0_d = nc.declare_dram_parameter("selfF0", [128, SHP], FP16, isOutput=False)
    Wh_d = [
        nc.declare_dram_parameter(f"Wh{j}", [128, DIMS[j][1]], FP16, isOutput=False)
        for j in range(6)
    ]  # W3 is row-duplicated [128, 128] on the host
    W3s_d = nc.declare_dram_parameter("W3s", [64, 128], FP16, isOutput=False)
    gb_d = [
        nc.declare_dram_parameter(f"gb{j}", [128, 2], FP32, isOutput=False)
        for j in range(6)
    ]
    out_d = nc.declare_dram_parameter("out", [128, SHP], FP32, isOutput=True)

    # ---- internal DRAM: collective bounce buffers ----
    ag_in = [
        nc.dram_tensor(f"ag_in{j}", [SHP, DIMS[j][1]], FP16) for j in range(5)
    ]
    ag_out = [
        nc.dram_tensor(f"ag_out{j}", [NP, DIMS[j][1]], FP16, addr_space="Shared")
        for j in range(5)
    ]
    ar_in = [nc.dram_tensor(f"ar_in{j}", [128, 2], FP32) for j in range(6)]
    ar_out = [
        nc.dram_tensor(f"ar_out{j}", [128, 2], FP32, addr_space="Shared")
        for j in range(6)
    ]

    tbls = [None] + ag_out  # layer 0 streams pre-gathered messages instead

    chunks_m = [_chunks_of(TA), _chunks_of(TB)]
    chunks_3 = [_chunks_of(TA, CHUNK3), _chunks_of(TB, CHUNK3)]
    idx_col0_m = [[], []]
    idx_col0_3 = [[], []]
    for h in (0, 1):
        c = 0
        for _, nt in chunks_m[h]:
            idx_col0_m[h].append(c)
            c += nt * 8
        c = 0
        for _, nt in chunks_3[h]:
            idx_col0_3[h].append(c)
            c += nt * 8

    inv_n = 1.0 / float(N)

    with tile.TileContext(nc) as tc:
        with (
            tc.tile_pool(name="res", bufs=1) as res,
            tc.tile_pool(name="msg", bufs=5) as msgp,
            tc.tile_pool(name="s2", bufs=3) as s2p,
            tc.tile_pool(name="small", bufs=2) as small,
            tc.tile_pool(name="big", bufs=1) as big,
            tc.tile_pool(name="fpp", bufs=2) as fpp,
            tc.tile_pool(name="hx", bufs=1) as hxp,
            tc.tile_pool(name="agg_ps", bufs=2, space="PSUM") as aggp,
            tc.tile_pool(name="y_ps", bufs=2, space="PSUM") as yp,
        ):
            # ---- resident loads ----
            idx_t = [res.tile([128, Ts[h] * 8], I16, name=f"idx{h}") for h in (0, 1)]
            idx3_t = [res.tile([128, Ts[h] * 8], I16, name=f"idx3{h}") for h in (0, 1)]
            for h in (0, 1):
                nc.sync.dma_start(idx_t[h][:], idx_d[h][:])
                nc.sync.dma_start(idx3_t[h][:], idx3_d[h][:])
            dinvh_t = res.tile([128, SHP], FP16, name="dinvh")
            nc.sync.dma_start(dinvh_t[:], dinvh_d[:])
            Wh_t = []
            for j in range(6):
                wt = res.tile([128, DIMS[j][1]], FP16, name=f"Wh{j}")
                nc.sync.dma_start(wt[:], Wh_d[j][:])
                Wh_t.append(wt)
            W3s_t = res.tile([64, 128], FP16, name="W3s")
            nc.sync.dma_start(W3s_t[:], W3s_d[:])
            gb_t = []
            for j in range(6):
                gt = res.tile([128, 2], FP32, name=f"gb{j}")
                nc.sync.dma_start(gt[:], gb_d[j][:])
                gb_t.append(gt)
            selfF = res.tile([128, SHP], FP16, name="selfF0")
            nc.sync.dma_start(selfF[:], selfF0_d[:])

            def bn_vec(j, fo, arr_tile):
                """mean/var -> (scale, shift) columns in a [128, 6] tile."""
                vec = small.tile([128, 6], FP32, tag="bnvec", name="vec")
                nc.vector.tensor_scalar(
                    out=vec[0:fo, 0:1], in0=arr_tile[0:fo, 0:1],
                    scalar1=inv_n, scalar2=None, op0=OP.mult,
                )
                nc.vector.tensor_scalar(
                    out=vec[0:fo, 1:2], in0=arr_tile[0:fo, 1:2],
                    scalar1=inv_n, scalar2=None, op0=OP.mult,
                )
                nc.vector.tensor_tensor(
                    vec[0:fo, 2:3], vec[0:fo, 0:1], vec[0:fo, 0:1], op=OP.mult
                )
                nc.vector.tensor_tensor(
                    vec[0:fo, 2:3], vec[0:fo, 1:2], vec[0:fo, 2:3], op=OP.subtract
                )
                nc.vector.tensor_scalar(
                    out=vec[0:fo, 2:3], in0=vec[0:fo, 2:3],
                    scalar1=float(EPS), scalar2=None, op0=OP.add,
                )
                nc.vector.reciprocal(vec[0:fo, 3:4], vec[0:fo, 2:3])
                nc.scalar.activation(vec[0:fo, 3:4], vec[0:fo, 3:4], ACTF.Sqrt)
                nc.vector.tensor_tensor(
                    vec[0:fo, 4:5], gb_t[j][0:fo, 0:1], vec[0:fo, 3:4], op=OP.mult
                )
                nc.vector.tensor_tensor(
                    vec[0:fo, 5:6], vec[0:fo, 0:1], vec[0:fo, 4:5], op=OP.mult
                )
                nc.vector.tensor_tensor(
                    vec[0:fo, 5:6], gb_t[j][0:fo, 1:2], vec[0:fo, 5:6], op=OP.subtract
                )
                return vec

            gq_ctr = [0]

            for j in range(DEBUG_NL):
                fo = DIMS[j][1]
                pair = j == 3
                tbl = tbls[j]
                idxs = idx3_t if pair else idx_t
                if pair:
                    tblv = tbl[:].rearrange("(a b) f -> a (b f)", b=2)
                s2src = s23_d if pair else s2_d

                cur_chunk = [-1, -1]
                msg_tiles = [None, None]
                s2_tiles = [None, None]
                chunks = chunks_3 if pair else chunks_m
                idx_col0 = idx_col0_3 if pair else idx_col0_m

                def ensure_chunk(h, t):
                    k = 0
                    while not (
                        chunks[h][k][0] <= t < chunks[h][k][0] + chunks[h][k][1]
                    ):
                        k += 1
                    if cur_chunk[h] == k:
                        return
                    cur_chunk[h] = k
                    t0c, ntc = chunks[h][k]
                    mt = msgp.tile([128, ntc, 128], FP16, tag="msg", name="msg")
                    if j == 0:
                        nc.sync.dma_start(
                            mt[:], msgs0_d[h][:, t0c : t0c + ntc, :]
                        )
                    else:
                        in_ap = tblv if pair else tbl[h * HALF : (h + 1) * HALF, :]
                        nc.gpsimd.dma_gather(
                            out_ap=mt[:],
                            in_ap=in_ap,
                            idxs_ap=idxs[h][
                                :, idx_col0[h][k] : idx_col0[h][k] + ntc * 8
                            ],
                            num_idxs=ntc * 128,
                            num_idxs_reg=ntc * 128,
                            elem_size=128,
                            single_packet=False,
                            queue_num=gq_ctr[0] % 4,
                        )
                        gq_ctr[0] += 1
                    msg_tiles[h] = (t0c, mt)
                    if pair:
                        st = s2p.tile([128, ntc, 2, 256], FP16, tag="s2", name="s2")
                        nc.sync.dma_start(st[:], s2src[h][:, t0c : t0c + ntc, :, :])
                    else:
                        st = s2p.tile([128, ntc, 256], FP16, tag="s2", name="s2")
                        nc.sync.dma_start(st[:], s2src[h][:, t0c : t0c + ntc, :])
                    s2_tiles[h] = (t0c, st)

                y_sb = big.tile([128, SHP], FP16, tag="ysb", name="ysb")
                sumP = small.tile([128, NSB], FP32, tag="sumP", name="sumP")
                sqP = small.tile([128, NSB], FP32, tag="sqP", name="sqP")
                junk = small.tile([128, 512], FP16, tag="junk", name="junk")

                for sb in range(NSB):
                    nsb = 512 if sb < 12 else 128
                    wplist = list(range(sb * 2, min(sb * 2 + 2, NWP)))
                    agg = aggp.tile([128, 512], FP32, tag="agg", name="agg")
                    for wp in wplist:
                        woff = (wp % 2) * 256
                        # sequence of (h, t, parity) 256-wide matmuls
                        seq = []
                        for h in (0, 1):
                            for t in range(t0s[wp][h], t0s[wp][h] + tiles[wp][h]):
                                if pair:
                                    seq.append((h, t, 0))
                                    seq.append((h, t, 1))
                                else:
                                    seq.append((h, t, None))
                        fi_eff = 64 if pair else 128
                        for i, (h, t, ps) in enumerate(seq):
                            ensure_chunk(h, t)
                            t0c, mt = msg_tiles[h]
                            s0c, st = s2_tiles[h]
                            if ps is not None:
                                lhsT = mt[:, t - t0c, 64 * ps : 64 * ps + 64]
                                rhs = st[:, t - s0c, ps, :]
                            else:
                                lhsT = mt[:, t - t0c, :]
                                rhs = st[:, t - s0c, :]
                            nc.tensor.matmul(
                                agg[0:fi_eff, woff : woff + 256],
                                lhsT,
                                rhs,
                                start=(i == 0),
                                stop=(i == len(seq) - 1),
                            )
                    # evict agg -> fp16
                    fi_eff = 64 if pair else 128
                    wmain = W3s_t if pair else Wh_t[j]
                    rawT = small.tile([128, 512], FP16, tag="rawT", name="rawT")
                    nc.scalar.activation(
                        rawT[0:fi_eff, 0:nsb], agg[0:fi_eff, 0:nsb], ACTF.Copy
                    )
                    # y = W^T agg + W^T selfF  (fp16 matmuls, f32 psum)
                    c0 = sb * 512
                    y_ps = yp.tile([128, 512], FP32, tag="yps", name="yps")
                    nc.tensor.matmul(
                        y_ps[0:fo, 0:nsb],
                        wmain[0:fi_eff, 0:fo],
                        rawT[0:fi_eff, 0:nsb],
                        start=True,
                        stop=False,
                    )
                    nc.tensor.matmul(
                        y_ps[0:fo, 0:nsb],
                        wmain[0:fi_eff, 0:fo],
                        selfF[0:fi_eff, c0 : c0 + nsb],
                        start=False,
                        stop=True,
                    )
                    # copy to y_sb + stats over valid columns
                    nv = 512 if sb < 12 else 106
                    nc.scalar.activation(
                        y_sb[0:fo, c0 : c0 + nv],
                        y_ps[0:fo, 0:nv],
                        ACTF.Copy,
                        accum_out=sumP[0:fo, sb : sb + 1],
                    )
                    if sb == 12:
                        nc.scalar.activation(
                            y_sb[0:fo, c0 + 106 : c0 + 128],
                            y_ps[0:fo, 106:128],
                            ACTF.Copy,
                        )
                    nc.scalar.activation(
                        junk[0:fo, 0:nv],
                        y_ps[0:fo, 0:nv],
                        ACTF.Square,
                        accum_out=sqP[0:fo, sb : sb + 1],
                    )

                # ---- kick BN stats all-reduce ----
                stats = small.tile([128, 2], FP32, tag="stats", name="stats")
                nc.vector.memset(stats[:], 0.0)
                nc.vector.reduce_sum(stats[0:fo, 0:1], sumP[0:fo, :], axis=AX)
                nc.vector.reduce_sum(stats[0:fo, 1:2], sqP[0:fo, :], axis=AX)
                nc.sync.dma_start(ar_in[j][:], stats[:])
                nc.gpsimd.collective_compute(
                    "AllReduce",
                    OP.add,
                    replica_groups=[list(range(NC))],
                    ins=[ar_in[j][:]],
                    outs=[ar_out[j][:]],
                )
                arr = small.tile([128, 2], FP32, tag="arr", name="arr")
                nc.sync.dma_start(arr[:], ar_out[j][:])
                vec = bn_vec(j, fo, arr)

                if j == 5 or j == DEBUG_NL - 1:
                    # final layer: BN apply fp16 -> f32 staging chunks, DMA out
                    for sb in range(NSB):
                        nsb = 512 if sb < 12 else 128
                        c0 = sb * 512
                        stg = small.tile([128, 512], FP32, tag="ostg", name="ostg")
                        nc.scalar.activation(
                            stg[0:fo, 0:nsb],
                            y_sb[0:fo, c0 : c0 + nsb],
                            ACTF.Identity,
                            bias=vec[0:fo, 5:6],
                            scale=vec[0:fo, 4:5],
                        )
                        nc.sync.dma_start(
                            out_d[:, c0 : c0 + nsb], stg[:, 0:nsb]
                        )
                    continue

                # ---- BN apply (+ReLU) feature-major -> fp16 ----
                y_bnh = fpp.tile([128, SHP], FP16, tag="fp", name="ybnh")
                fn = ACTF.Relu if RELU[j] else ACTF.Identity
                for sb in range(NSB):
                    nsb = 512 if sb < 12 else 128
                    c0 = sb * 512
                    nc.scalar.activation(
                        y_bnh[0:fo, c0 : c0 + nsb],
                        y_sb[0:fo, c0 : c0 + nsb],
                        fn,
                        bias=vec[0:fo, 5:6],
                        scale=vec[0:fo, 4:5],
                    )
                # selfF_next = y_bn * dinv^2 (src scale is folded into S2)
                selfF_next = fpp.tile([128, SHP], FP16, tag="fp", name="selfFn")
                nc.vector.tensor_tensor(
                    selfF_next[0:fo, :], y_bnh[0:fo, :], dinvh_t[0:fo, :], op=OP.mult
                )
                # xbar transpose -> node-major [128, NWIN, fo]
                hnext = hxp.tile([128, NWIN, fo], FP16, tag="hx", name="hnext")
                nc.sync.dma_start_transpose(hnext[:], y_bnh[0:fo, :])
                nc.sync.dma_start(
                    ag_in[j][:].rearrange("(b p) f -> p b f", p=128), hnext[:]
                )
                nc.gpsimd.collective_compute(
                    "AllGather",
                    OP.bypass,
                    replica_groups=[list(range(NC))],
                    ins=[ag_in[j][:]],
                    outs=[ag_out[j][:]],
                )
                selfF = selfF_next

    nc.compile()
    return nc


def kernel(x, edge_index, **params):
    global LAST_RESULT

    from concourse.bass_utils import run_bass_kernel_spmd

    x = np.asarray(x, np.float32)
    edge_index = np.asarray(edge_index, np.int64)
    src_all = edge_index[0]
    dst_all = edge_index[1]

    deg = (np.bincount(dst_all, minlength=N) + 1.0).astype(np.float32)
    dinv = (1.0 / np.sqrt(deg)).astype(np.float32)

    # padded-layout helpers
    remap = (src_all // SH) * SHP + (src_all % SH)

    # padded raw x (layer-0 messages; src scale lives in S2) + x*dinv for selfF0
    hs0 = np.zeros((NP, F_IN), np.float32)
    x_pad_h = np.zeros((NP, F_IN), np.float16)
    xs = x * dinv[:, None]
    dinvp = np.zeros(NP, np.float32)  # dinv per padded id
    for c in range(NC):
        hs0[c * SHP : c * SHP + SH] = xs[c * SH : (c + 1) * SH]
        x_pad_h[c * SHP : c * SHP + SH] = x[c * SH : (c + 1) * SH]
        dinvp[c * SHP : c * SHP + SH] = dinv[c * SH : (c + 1) * SH]

    # per-core edge lists
    eds = []
    dinv_dst = []
    for c in range(NC):
        m = (dst_all >= c * SH) & (dst_all < (c + 1) * SH)
        dstl = dst_all[m] - c * SH
        srcr = remap[m]
        eds.append(_prep_edges(srcr, dstl))
        dv = np.zeros(SHP, np.float32)
        dv[:SH] = dinv[c * SH : (c + 1) * SH]
        dinv_dst.append(dv)

    tiles = [[0, 0] for _ in range(NWP)]
    for w in range(NWP):
        for h in (0, 1):
            mx = max(len(eds[c][w][h][0]) for c in range(NC))
            tiles[w][h] = -(-mx // 128) if mx else 0
    t0s = [[0, 0] for _ in range(NWP)]
    ta = tb = 0
    for w in range(NWP):
        t0s[w][0] = ta
        ta += tiles[w][0]
        t0s[w][1] = tb
        tb += tiles[w][1]
    TA, TB = ta, tb
    Ts = [TA, TB]

    chunksM = [_chunks_of(TA), _chunks_of(TB)]
    chunks3 = [_chunks_of(TA, CHUNK3), _chunks_of(TB, CHUNK3)]

    in_maps = []
    for c in range(NC):
        # build per-half packed streams
        idx_h, idx3_h, s2_h, s23_h, m0_h = [], [], [], [], []
        for h in (0, 1):
            T = Ts[h]
            gidx = np.zeros(T * 128, np.int16)  # half-local src id
            gidx3 = np.zeros(T * 128, np.int16)  # pair id (global)
            s2f = np.zeros((T * 128, 256), np.float32)
            s23f = np.zeros((T * 128, 2, 256), np.float32)
            m0 = np.zeros((T * 128, 128), np.float16)
            for w in range(NWP):
                gu, inv, r = eds[c][w][h]
                nt = tiles[w][h]
                n = len(gu)
                assert n <= nt * 128
                base = t0s[w][h] * 128
                rows = base + np.arange(n)
                gidx[rows] = (gu - h * HALF).astype(np.int16)
                gidx3[rows] = (gu // 2).astype(np.int16)
                vals = dinv_dst[c][w * WP + r] * dinvp[gu[inv]]
                erows = base + inv  # per-edge slot row
                np.add.at(s2f, (erows, r), vals)
                np.add.at(s23f, (erows, (gu[inv] % 2), r), vals)
                m0[rows] = x_pad_h[gu]
            idx_h.append(_wrap_idx(gidx, chunksM[h]))
            idx3_h.append(_wrap_idx(gidx3, chunks3[h]))
            # pre-wrapped layouts [128, T, ...]: partition = slot in tile
            s2_h.append(
                np.ascontiguousarray(
                    s2f.astype(np.float16).reshape(T, 128, 256).transpose(1, 0, 2)
                )
            )
            s23_h.append(
                np.ascontiguousarray(
                    s23f.astype(np.float16)
                    .reshape(T, 128, 2, 256)
                    .transpose(1, 0, 2, 3)
                )
            )
            m0_h.append(
                np.ascontiguousarray(m0.reshape(T, 128, 128).transpose(1, 0, 2))
            )

        dinvT = dinv_dst[c]
        dinv2Th = np.broadcast_to(
            (dinvT * dinvT).astype(np.float16), (128, SHP)
        ).copy()
        own = hs0[c * SHP : (c + 1) * SHP]  # [SHP, F] f32 (= x*dinv)
        selfF0 = (own * dinvT[:, None]).T.astype(np.float16).copy()  # [F, SHP]

        im = {
            "msgs0A": m0_h[0],
            "msgs0B": m0_h[1],
            "idxA": idx_h[0],
            "idxB": idx_h[1],
            "idx3A": idx3_h[0],
            "idx3B": idx3_h[1],
            "s2A": s2_h[0],
            "s2B": s2_h[1],
            "s23A": s23_h[0],
            "s23B": s23_h[1],
            "dinv2Th": dinv2Th,
            "selfF0": selfF0,
        }
        for j in range(6):
            W = np.asarray(params[f"W{j}"], np.float32)
            if j == 3:
                Wd = np.vstack([W, W])  # [128, 128]
            else:
                Wd = W
                if Wd.shape[0] < 128:
                    Wd = np.vstack([Wd, np.zeros((128 - Wd.shape[0], Wd.shape[1]), np.float32)])
            im[f"Wh{j}"] = Wd.astype(np.float16)
            gb = np.zeros((128, 2), np.float32)
            fo = DIMS[j][1]
            gb[:fo, 0] = np.asarray(params[f"g{j}"], np.float32)
            gb[:fo, 1] = np.asarray(params[f"be{j}"], np.float32)
            im[f"gb{j}"] = gb
        im["W3s"] = np.asarray(params["W3"], np.float32).astype(np.float16)
        in_maps.append(im)

    nc = _build_program(tiles, t0s, TA, TB)
    res = run_bass_kernel_spmd(
        nc,
        in_maps,
        core_ids=list(range(NC)),
        trace=TRACE,
        **TRACE_KW,
    )
    LAST_RESULT = res

    out = np.empty((N, F_IN), np.float32)
    for c in range(NC):
        out[c * SH : (c + 1) * SH] = res.results[c]["out"].T[:SH]
    return out



# revision 5
# speedup vs baseline: 1.8410x; 1.0094x over previous
"""Trainium2 Bass kernel for nn_AutoEncoder (6-layer GCN autoencoder).

v5 = v4 + de-paired layer 3: the layer-3 table is post2 duplicated to
128 columns (two DMA writes of the same transposed slab), so layer 3 uses
the regular idx/S2 path with single 64-row matmuls — no pair gathers, no
idx3/s23 streams.

v2 + multi-queue gathers (HW 3.26 ms, was 5.02 ms): dma_gather descriptor
generation runs on Q7 core pair (2*queue_num, 2*queue_num+1), so rotating
gathers across 4 SWDGE queues overlaps desc-gen ~2.5x. msg pool bufs=5
keeps >=4 gather chunks in flight.

Strategy (8 NeuronCores, SPMD), v2:
  - Destination nodes sharded across cores (6250/core, padded to 6272).
  - Node features replicated per layer via AllGather into a padded fp16
    [8*6272, F] node-major DRAM table; per-core dma_gather of h[src] for this
    core's edges (edge lists sorted by local dst window, split into two
    int16-index halves).
  - Segment-sum via one-hot matmuls accumulated in PSUM per 512-dst
    superblock. The one-hot S matrices are HOST-precomputed (fp16, with the
    dst-side deg^-1/2 scale folded into the values) and streamed from DRAM —
    no on-chip one-hot builds.
  - Layer 3 (64-wide input) gathers fp16 node-PAIR rows (256 B) from the
    64-wide table viewed as [NP/2, 128]; parity-split S matrices route each
    edge's correct half, and W3 rows are duplicated so the doubled agg rows
    sum back. All gather tables are therefore uniform 128-wide fp16.
  - The self-loop term never touches the edge path: selfF = y_bn * dinv^2
    stays feature-major and enters the y PSUM via extra fp16 W-matmuls.
  - Everything stays feature-major: BN stats accumulate on ACT during PSUM
    eviction, BN apply (+ReLU) is an ACT pass with per-partition scale/bias,
    and the next-layer node-major table is produced by a DMA xbar transpose
    (no PE transposes, no node-major DVE work).
  - The GCN bias b is skipped (training-mode BatchNorm absorbs it).
"""

import sys

sys.path.insert(0, "/opt/trn_rl_repo")

import numpy as np

N = 50000
E = 800000
F_IN = 128
EPS = 1e-5
NC = 8
SH = 6250  # real dst nodes per core
SHP = 6272  # padded (49 * 128)
NP = NC * SHP  # 50176 rows in the padded replicated node table
HALF = NP // 2  # 25088 (< int16 max) rows per gather table half
WIN = 128  # dst window = psum column band
NWIN = SHP // WIN  # 49
WP = 256  # tile at window-PAIR granularity; scatter matmuls are 256 wide
NWP = 25  # 24 full pairs + lone window 48
NSB = 13  # psum superblocks: 12 x 512 + 1 x 128
CHUNK = 32  # gather chunk size in tiles of 128 edges (main layers)
CHUNK3 = 16  # layer-3 chunk (S2 tiles are 4x wide there)
DIMS = [(128, 128), (128, 128), (128, 64), (64, 128), (128, 128), (128, 128)]
RELU = [True, True, False, True, True, False]

import os as _os
DEBUG_NL = int(_os.environ.get("DEBUG_NL", "6"))  # layers to run (debug)
TRACE = False  # set by test.py for profiling runs
TRACE_KW = {}
LAST_RESULT = None  # BassKernelResults of the last run (for test.py)


def _prep_edges(src_remap, dstl):
    """Per-core edge prep: sort by dst; per-(window, half) edge lists."""
    order = np.argsort(dstl, kind="stable")
    dstl = dstl[order]
    srcr = src_remap[order]
    half = (srcr >= HALF).astype(np.int64)
    w = dstl // WP
    rel = dstl - w * WP
    ed = [[None, None] for _ in range(NWP)]
    for wi in range(NWP):
        m = w == wi
        for h in (0, 1):
            mh = m & (half == h)
            # dedupe by src: one gather slot per distinct src in this window
            gu, inv = np.unique(srcr[mh], return_inverse=True)
            ed[wi][h] = (gu, inv, rel[mh])
    return ed


def _chunks_of(T, ck=CHUNK):
    out = []
    t = 0
    while t < T:
        nt = min(ck, T - t)
        out.append((t, nt))
        t += nt
    return out


def _wrap_idx(gidx, chunks):
    """int16 indices -> [128, total/16] wrapped per chunk, tiled 8x."""
    total_cols = len(gidx) // 16
    arr = np.zeros((16, total_cols), np.int16)
    col = 0
    for t0, nt in chunks:
        cidx = gidx[t0 * 128 : (t0 + nt) * 128]
        ncol = len(cidx) // 16
        arr[:, col : col + ncol] = cidx.reshape(ncol, 16).T
        col += ncol
    assert col == total_cols
    return np.tile(arr, (8, 1)).copy()


def _build_program(tiles, t0s, TA, TB):
    from concourse import bacc, mybir, tile

    FP32 = mybir.dt.float32
    FP16 = mybir.dt.float16
    I16 = mybir.dt.int16
    AX = mybir.AxisListType.X
    OP = mybir.AluOpType
    ACTF = mybir.ActivationFunctionType

    nc = bacc.Bacc(None, num_devices=NC, target_bir_lowering=False, debug=False, num_swdge_queues=4)

    Ts = [TA, TB]

    # ---- parameters ----
    msgs0_d = [
        nc.declare_dram_parameter("msgs0A", [128, TA, 128], FP16, isOutput=False),
        nc.declare_dram_parameter("msgs0B", [128, TB, 128], FP16, isOutput=False),
    ]
    idx_d = [
        nc.declare_dram_parameter("idxA", [128, TA * 8], I16, isOutput=False),
        nc.declare_dram_parameter("idxB", [128, TB * 8], I16, isOutput=False),
    ]
    s2_d = [
        nc.declare_dram_parameter("s2A", [128, TA, 256], FP16, isOutput=False),
        nc.declare_dram_parameter("s2B", [128, TB, 256], FP16, isOutput=False),
    ]
    dinvh_d = nc.declare_dram_parameter("dinv2Th", [128, SHP], FP16, isOutput=False)
    selfF0_d = nc.declare_dram_parameter("selfF0", [128, SHP], FP16, isOutput=False)
    Wh_d = [
        nc.declare_dram_parameter(f"Wh{j}", [128, DIMS[j][1]], FP16, isOutput=False)
        for j in range(6)
    ]  # W3 is row-duplicated [128, 128] on the host
    W3s_d = nc.declare_dram_parameter("W3s", [64, 128], FP16, isOutput=False)
    gb_d = [
        nc.declare_dram_parameter(f"gb{j}", [128, 2], FP32, isOutput=False)
        for j in range(6)
    ]
    out_d = nc.declare_dram_parameter("out", [128, SHP], FP32, isOutput=True)

    # ---- internal DRAM: collective bounce buffers ----
    TBL_W = [DIMS[j][1] for j in range(5)]
    TBL_W[2] = 128  # layer-3 table carries post2 duplicated to 128 cols
    ag_in = [
        nc.dram_tensor(f"ag_in{j}", [SHP, TBL_W[j]], FP16) for j in range(5)
    ]
    ag_out = [
        nc.dram_tensor(f"ag_out{j}", [NP, TBL_W[j]], FP16, addr_space="Shared")
        for j in range(5)
    ]
    ar_in = [nc.dram_tensor(f"ar_in{j}", [128, 2], FP32) for j in range(6)]
    ar_out = [
        nc.dram_tensor(f"ar_out{j}", [128, 2], FP32, addr_space="Shared")
        for j in range(6)
    ]

    tbls = [None] + ag_out  # layer 0 streams pre-gathered messages instead

    chunks_m = [_chunks_of(TA), _chunks_of(TB)]
    idx_col0_m = [[], []]
    for h in (0, 1):
        c = 0
        for _, nt in chunks_m[h]:
            idx_col0_m[h].append(c)
            c += nt * 8

    inv_n = 1.0 / float(N)

    with tile.TileContext(nc) as tc:
        with (
            tc.tile_pool(name="res", bufs=1) as res,
            tc.tile_pool(name="msg", bufs=5) as msgp,
            tc.tile_pool(name="s2", bufs=3) as s2p,
            tc.tile_pool(name="small", bufs=2) as small,
            tc.tile_pool(name="big", bufs=1) as big,
            tc.tile_pool(name="fpp", bufs=2) as fpp,
            tc.tile_pool(name="hx", bufs=1) as hxp,
            tc.tile_pool(name="agg_ps", bufs=2, space="PSUM") as aggp,
            tc.tile_pool(name="y_ps", bufs=2, space="PSUM") as yp,
        ):
            # ---- resident loads ----
            idx_t = [res.tile([128, Ts[h] * 8], I16, name=f"idx{h}") for h in (0, 1)]
            for h in (0, 1):
                nc.sync.dma_start(idx_t[h][:], idx_d[h][:])
            dinvh_t = res.tile([128, SHP], FP16, name="dinvh")
            nc.sync.dma_start(dinvh_t[:], dinvh_d[:])
            Wh_t = []
            for j in range(6):
                wt = res.tile([128, DIMS[j][1]], FP16, name=f"Wh{j}")
                nc.sync.dma_start(wt[:], Wh_d[j][:])
                Wh_t.append(wt)
            W3s_t = res.tile([64, 128], FP16, name="W3s")
            nc.sync.dma_start(W3s_t[:], W3s_d[:])
            gb_t = []
            for j in range(6):
                gt = res.tile([128, 2], FP32, name=f"gb{j}")
                nc.sync.dma_start(gt[:], gb_d[j][:])
                gb_t.append(gt)
            selfF = res.tile([128, SHP], FP16, name="selfF0")
            nc.sync.dma_start(selfF[:], selfF0_d[:])

            def bn_vec(j, fo, arr_tile):
                """mean/var -> (scale, shift) columns in a [128, 6] tile."""
                vec = small.tile([128, 6], FP32, tag="bnvec", name="vec")
                nc.vector.tensor_scalar(
                    out=vec[0:fo, 0:1], in0=arr_tile[0:fo, 0:1],
                    scalar1=inv_n, scalar2=None, op0=OP.mult,
                )
                nc.vector.tensor_scalar(
                    out=vec[0:fo, 1:2], in0=arr_tile[0:fo, 1:2],
                    scalar1=inv_n, scalar2=None, op0=OP.mult,
                )
                nc.vector.tensor_tensor(
                    vec[0:fo, 2:3], vec[0:fo, 0:1], vec[0:fo, 0:1], op=OP.mult
                )
                nc.vector.tensor_tensor(
                    vec[0:fo, 2:3], vec[0:fo, 1:2], vec[0:fo, 2:3], op=OP.subtract
                )
                nc.vector.tensor_scalar(
                    out=vec[0:fo, 2:3], in0=vec[0:fo, 2:3],
                    scalar1=float(EPS), scalar2=None, op0=OP.add,
                )
                nc.vector.reciprocal(vec[0:fo, 3:4], vec[0:fo, 2:3])
                nc.scalar.activation(vec[0:fo, 3:4], vec[0:fo, 3:4], ACTF.Sqrt)
                nc.vector.tensor_tensor(
                    vec[0:fo, 4:5], gb_t[j][0:fo, 0:1], vec[0:fo, 3:4], op=OP.mult
                )
                nc.vector.tensor_tensor(
                    vec[0:fo, 5:6], vec[0:fo, 0:1], vec[0:fo, 4:5], op=OP.mult
                )
                nc.vector.tensor_tensor(
                    vec[0:fo, 5:6], gb_t[j][0:fo, 1:2], vec[0:fo, 5:6], op=OP.subtract
                )
                return vec

            gq_ctr = [0]

            for j in range(DEBUG_NL):
                fo = DIMS[j][1]
                fi_eff = 64 if j == 3 else 128
                tbl = tbls[j]
                idxs = idx_t
                s2src = s2_d

                cur_chunk = [-1, -1]
                msg_tiles = [None, None]
                s2_tiles = [None, None]
                chunks = chunks_m
                idx_col0 = idx_col0_m

                def ensure_chunk(h, t):
                    k = 0
                    while not (
                        chunks[h][k][0] <= t < chunks[h][k][0] + chunks[h][k][1]
                    ):
                        k += 1
                    if cur_chunk[h] == k:
                        return
                    cur_chunk[h] = k
                    t0c, ntc = chunks[h][k]
                    mt = msgp.tile([128, ntc, 128], FP16, tag="msg", name="msg")
                    if j == 0:
                        nc.sync.dma_start(
                            mt[:], msgs0_d[h][:, t0c : t0c + ntc, :]
                        )
                    else:
                        in_ap = tbl[h * HALF : (h + 1) * HALF, :]
                        nc.gpsimd.dma_gather(
                            out_ap=mt[:],
                            in_ap=in_ap,
                            idxs_ap=idxs[h][
                                :, idx_col0[h][k] : idx_col0[h][k] + ntc * 8
                            ],
                            num_idxs=ntc * 128,
                            num_idxs_reg=ntc * 128,
                            elem_size=128,
                            single_packet=False,
                            queue_num=gq_ctr[0] % 4,
                        )
                        gq_ctr[0] += 1
                    msg_tiles[h] = (t0c, mt)
                    st = s2p.tile([128, ntc, 256], FP16, tag="s2", name="s2")
                    nc.sync.dma_start(st[:], s2src[h][:, t0c : t0c + ntc, :])
                    s2_tiles[h] = (t0c, st)

                y_sb = big.tile([128, SHP], FP16, tag="ysb", name="ysb")
                sumP = small.tile([128, NSB], FP32, tag="sumP", name="sumP")
                sqP = small.tile([128, NSB], FP32, tag="sqP", name="sqP")
                junk = small.tile([128, 512], FP16, tag="junk", name="junk")

                for sb in range(NSB):
                    nsb = 512 if sb < 12 else 128
                    wplist = list(range(sb * 2, min(sb * 2 + 2, NWP)))
                    agg = aggp.tile([128, 512], FP32, tag="agg", name="agg")
                    for wp in wplist:
                        woff = (wp % 2) * 256
                        seq = []
                        for h in (0, 1):
                            for t in range(t0s[wp][h], t0s[wp][h] + tiles[wp][h]):
                                seq.append((h, t))
                        for i, (h, t) in enumerate(seq):
                            ensure_chunk(h, t)
                            t0c, mt = msg_tiles[h]
                            s0c, st = s2_tiles[h]
                            nc.tensor.matmul(
                                agg[0:fi_eff, woff : woff + 256],
                                mt[:, t - t0c, 0:fi_eff],
                                st[:, t - s0c, :],
                                start=(i == 0),
                                stop=(i == len(seq) - 1),
                            )
                    # evict agg -> fp16
                    wmain = W3s_t if j == 3 else Wh_t[j]
                    rawT = small.tile([128, 512], FP16, tag="rawT", name="rawT")
                    nc.scalar.activation(
                        rawT[0:fi_eff, 0:nsb], agg[0:fi_eff, 0:nsb], ACTF.Copy
                    )
                    # y = W^T agg + W^T selfF  (fp16 matmuls, f32 psum)
                    c0 = sb * 512
                    y_ps = yp.tile([128, 512], FP32, tag="yps", name="yps")
                    nc.tensor.matmul(
                        y_ps[0:fo, 0:nsb],
                        wmain[0:fi_eff, 0:fo],
                        rawT[0:fi_eff, 0:nsb],
                        start=True,
                        stop=False,
                    )
                    nc.tensor.matmul(
                        y_ps[0:fo, 0:nsb],
                        wmain[0:fi_eff, 0:fo],
                        selfF[0:fi_eff, c0 : c0 + nsb],
                        start=False,
                        stop=True,
                    )
                    # copy to y_sb + stats over valid columns
                    nv = 512 if sb < 12 else 106
                    nc.scalar.activation(
                        y_sb[0:fo, c0 : c0 + nv],
                        y_ps[0:fo, 0:nv],
                        ACTF.Copy,
                        accum_out=sumP[0:fo, sb : sb + 1],
                    )
                    if sb == 12:
                        nc.scalar.activation(
                            y_sb[0:fo, c0 + 106 : c0 + 128],
                            y_ps[0:fo, 106:128],
                            ACTF.Copy,
                        )
                    nc.scalar.activation(
                        junk[0:fo, 0:nv],
                        y_ps[0:fo, 0:nv],
                        ACTF.Square,
                        accum_out=sqP[0:fo, sb : sb + 1],
                    )

                # ---- kick BN stats all-reduce ----
                stats = small.tile([128, 2], FP32, tag="stats", name="stats")
                nc.vector.memset(stats[:], 0.0)
                nc.vector.reduce_sum(stats[0:fo, 0:1], sumP[0:fo, :], axis=AX)
                nc.vector.reduce_sum(stats[0:fo, 1:2], sqP[0:fo, :], axis=AX)
                nc.sync.dma_start(ar_in[j][:], stats[:])
                nc.gpsimd.collective_compute(
                    "AllReduce",
                    OP.add,
                    replica_groups=[list(range(NC))],
                    ins=[ar_in[j][:]],
                    outs=[ar_out[j][:]],
                )
                arr = small.tile([128, 2], FP32, tag="arr", name="arr")
                nc.sync.dma_start(arr[:], ar_out[j][:])
                vec = bn_vec(j, fo, arr)

                if j == 5 or j == DEBUG_NL - 1:
                    # final layer: BN apply fp16 -> f32 staging chunks, DMA out
                    for sb in range(NSB):
                        nsb = 512 if sb < 12 else 128
                        c0 = sb * 512
                        stg = small.tile([128, 512], FP32, tag="ostg", name="ostg")
                        nc.scalar.activation(
                            stg[0:fo, 0:nsb],
                            y_sb[0:fo, c0 : c0 + nsb],
                            ACTF.Identity,
                            bias=vec[0:fo, 5:6],
                            scale=vec[0:fo, 4:5],
                        )
                        nc.sync.dma_start(
                            out_d[:, c0 : c0 + nsb], stg[:, 0:nsb]
                        )
                    continue

                # ---- BN apply (+ReLU) feature-major -> fp16 ----
                y_bnh = fpp.tile([128, SHP], FP16, tag="fp", name="ybnh")
                fn = ACTF.Relu if RELU[j] else ACTF.Identity
                for sb in range(NSB):
                    nsb = 512 if sb < 12 else 128
                    c0 = sb * 512
                    nc.scalar.activation(
                        y_bnh[0:fo, c0 : c0 + nsb],
                        y_sb[0:fo, c0 : c0 + nsb],
                        fn,
                        bias=vec[0:fo, 5:6],
                        scale=vec[0:fo, 4:5],
                    )
                # selfF_next = y_bn * dinv^2 (src scale is folded into S2)
                selfF_next = fpp.tile([128, SHP], FP16, tag="fp", name="selfFn")
                nc.vector.tensor_tensor(
                    selfF_next[0:fo, :], y_bnh[0:fo, :], dinvh_t[0:fo, :], op=OP.mult
                )
                # xbar transpose -> node-major [128, NWIN, fo]
                hnext = hxp.tile([128, NWIN, fo], FP16, tag="hx", name="hnext")
                nc.sync.dma_start_transpose(hnext[:], y_bnh[0:fo, :])
                if j == 2:
                    nc.sync.dma_start(
                        ag_in[j][:, 0:64].rearrange("(b p) f -> p b f", p=128),
                        hnext[:],
                    )
                    nc.sync.dma_start(
                        ag_in[j][:, 64:128].rearrange("(b p) f -> p b f", p=128),
                        hnext[:],
                    )
                else:
                    nc.sync.dma_start(
                        ag_in[j][:].rearrange("(b p) f -> p b f", p=128), hnext[:]
                    )
                nc.gpsimd.collective_compute(
                    "AllGather",
                    OP.bypass,
                    replica_groups=[list(range(NC))],
                    ins=[ag_in[j][:]],
                    outs=[ag_out[j][:]],
                )
                selfF = selfF_next

    nc.compile()
    return nc


def kernel(x, edge_index, **params):
    global LAST_RESULT

    from concourse.bass_utils import run_bass_kernel_spmd

    x = np.asarray(x, np.float32)
    edge_index = np.asarray(edge_index, np.int64)
    src_all = edge_index[0]
    dst_all = edge_index[1]

    deg = (np.bincount(dst_all, minlength=N) + 1.0).astype(np.float32)
    dinv = (1.0 / np.sqrt(deg)).astype(np.float32)

    # padded-layout helpers
    remap = (src_all // SH) * SHP + (src_all % SH)

    # padded raw x (layer-0 messages; src scale lives in S2) + x*dinv for selfF0
    hs0 = np.zeros((NP, F_IN), np.float32)
    x_pad_h = np.zeros((NP, F_IN), np.float16)
    xs = x * dinv[:, None]
    dinvp = np.zeros(NP, np.float32)  # dinv per padded id
    for c in range(NC):
        hs0[c * SHP : c * SHP + SH] = xs[c * SH : (c + 1) * SH]
        x_pad_h[c * SHP : c * SHP + SH] = x[c * SH : (c + 1) * SH]
        dinvp[c * SHP : c * SHP + SH] = dinv[c * SH : (c + 1) * SH]

    # per-core edge lists
    eds = []
    dinv_dst = []
    for c in range(NC):
        m = (dst_all >= c * SH) & (dst_all < (c + 1) * SH)
        dstl = dst_all[m] - c * SH
        srcr = remap[m]
        eds.append(_prep_edges(srcr, dstl))
        dv = np.zeros(SHP, np.float32)
        dv[:SH] = dinv[c * SH : (c + 1) * SH]
        dinv_dst.append(dv)

    tiles = [[0, 0] for _ in range(NWP)]
    for w in range(NWP):
        for h in (0, 1):
            mx = max(len(eds[c][w][h][0]) for c in range(NC))
            tiles[w][h] = -(-mx // 128) if mx else 0
    t0s = [[0, 0] for _ in range(NWP)]
    ta = tb = 0
    for w in range(NWP):
        t0s[w][0] = ta
        ta += tiles[w][0]
        t0s[w][1] = tb
        tb += tiles[w][1]
    TA, TB = ta, tb
    Ts = [TA, TB]

    chunksM = [_chunks_of(TA), _chunks_of(TB)]

    in_maps = []
    for c in range(NC):
        # build per-half packed streams
        idx_h, s2_h, m0_h = [], [], []
        for h in (0, 1):
            T = Ts[h]
            gidx = np.zeros(T * 128, np.int16)  # half-local src id
            s2f = np.zeros((T * 128, 256), np.float32)
            m0 = np.zeros((T * 128, 128), np.float16)
            for w in range(NWP):
                gu, inv, r = eds[c][w][h]
                nt = tiles[w][h]
                n = len(gu)
                assert n <= nt * 128
                base = t0s[w][h] * 128
                rows = base + np.arange(n)
                gidx[rows] = (gu - h * HALF).astype(np.int16)
                vals = dinv_dst[c][w * WP + r] * dinvp[gu[inv]]
                erows = base + inv  # per-edge slot row
                np.add.at(s2f, (erows, r), vals)
                m0[rows] = x_pad_h[gu]
            idx_h.append(_wrap_idx(gidx, chunksM[h]))
            # pre-wrapped layouts [128, T, ...]: partition = slot in tile
            s2_h.append(
                np.ascontiguousarray(
                    s2f.astype(np.float16).reshape(T, 128, 256).transpose(1, 0, 2)
                )
            )
            m0_h.append(
                np.ascontiguousarray(m0.reshape(T, 128, 128).transpose(1, 0, 2))
            )

        dinvT = dinv_dst[c]
        dinv2Th = np.broadcast_to(
            (dinvT * dinvT).astype(np.float16), (128, SHP)
        ).copy()
        own = hs0[c * SHP : (c + 1) * SHP]  # [SHP, F] f32 (= x*dinv)
        selfF0 = (own * dinvT[:, None]).T.astype(np.float16).copy()  # [F, SHP]

        im = {
            "msgs0A": m0_h[0],
            "msgs0B": m0_h[1],
            "idxA": idx_h[0],
            "idxB": idx_h[1],
            "s2A": s2_h[0],
            "s2B": s2_h[1],
            "dinv2Th": dinv2Th,
            "selfF0": selfF0,
        }
        for j in range(6):
            W = np.asarray(params[f"W{j}"], np.float32)
            if j == 3:
                Wd = np.vstack([W, W])  # [128, 128]
            else:
                Wd = W
                if Wd.shape[0] < 128:
                    Wd = np.vstack([Wd, np.zeros((128 - Wd.shape[0], Wd.shape[1]), np.float32)])
            im[f"Wh{j}"] = Wd.astype(np.float16)
            gb = np.zeros((128, 2), np.float32)
            fo = DIMS[j][1]
            gb[:fo, 0] = np.asarray(params[f"g{j}"], np.float32)
            gb[:fo, 1] = np.asarray(params[f"be{j}"], np.float32)
            im[f"gb{j}"] = gb
        im["W3s"] = np.asarray(params["W3"], np.float32).astype(np.float16)
        in_maps.append(im)

    nc = _build_program(tiles, t0s, TA, TB)
    res = run_bass_kernel_spmd(
        nc,
        in_maps,
        core_ids=list(range(NC)),
        trace=TRACE,
        **TRACE_KW,
    )
    LAST_RESULT = res

    out = np.empty((N, F_IN), np.float32)
    for c in range(NC):
        out[c * SH : (c + 1) * SH] = res.results[c]["out"].T[:SH]
    return out

